# revision 1
# baseline (speedup 1.0000x reference)
"""GCN (4-layer, improved self-loops) on 8 Trainium2 NeuronCores.

Sharding: 1D node partition (6250 nodes/core); edges partitioned by
destination-node owner; per layer the prescaled features t_hat = dinv * (h@Wg)
are AllGathered into a full [50000, 128] DRAM table on every core, then each
core gathers per-edge source rows with dma_gather and scatter-adds them into
per-destination-block PSUM tiles via one-hot matmuls on the TensorEngine.

Self-loops (weight 2.0) are explicit edges, so the aggregation is one uniform
edge stream:
    h_next = elu(dinv[d] * sum_e w_e * t_hat[src_e] + b),  t_hat = dinv * t
which matches the reference exactly (norm_e = w_e * dinv[src] * dinv[dst] and
self_coef = 2 * dinv^2 both factor into the dinv sandwich).  deg/dinv are
O(E) scalar preprocessing computed on the host along with the edge partition.

On-chip node features are feature-major [H=128 partitions, nodes free]; the
aggregation matmul  PSUM[H, dst256] += V_tile^T @ Sw  (lhsT = gathered edge
rows, rhs = one-hot(dst_rel) * w built on the vector engine from iota +
metadata) lands feature-major again, so no transposes are needed between
layers.  Scatter/dense matmuls run as float32r (single-pass fp32) for 4x PE
row rate at moving dim >= 256.  Gather indices are int16, so the node table
is split at row 32768 into lo/hi streams.
"""

import numpy as np
from contextlib import ExitStack

try:
    import concourse.bass as bass
except ImportError:  # pragma: no cover
    import sys

    sys.path.insert(0, "/opt/trn_rl_repo")
    import concourse.bass as bass

import concourse.bacc as bacc
import concourse.mybir as mybir
import concourse.tile as tile
from concourse.bass_utils import run_bass_kernel_spmd

FP = mybir.dt.float32
FPR = mybir.dt.float32r
BF = mybir.dt.bfloat16
I16 = mybir.dt.int16

N = 50000
E = 800000
IN_D = 64
H = 128
OUT_D = 16
P = 8
NC_N = N // P            # 6250 nodes per core
BW = 256                 # destination-block width (scatter matmul moving dim)
NBLK = 25                # destination blocks per core (NPAD / BW)
NPAD = NBLK * BW         # 6400
NB_NODE = NPAD // 128    # 50 node-blocks of 128 for the t_hat path
SPLIT = 32768            # lo/hi src split so gather indices fit in int16
C_TILES = 32             # 128-edge tiles per dma_gather call
MC = 64                  # tiles per metadata DMA chunk

# dense-matmul column chunks over the padded node dim
CHUNKS = [(k * 512, 512) for k in range(12)] + [(6144, 256)]

ALU = mybir.AluOpType
ACT_F = mybir.ActivationFunctionType


def _prep_edges(edge_index, edge_weight):
    """Host preprocessing: partition edges by dst owner, add self loops,
    compute deg/dinv, split lo/hi by src, group by BW-dst block, pad each
    (core, block, stream) group to a common (max-over-cores) tile count.

    Returns (tlo, thi, per_core); per_core[c] has loidx/hiidx/meta/dinvrep/
    dinvT arrays.
    """
    src = np.asarray(edge_index[0], dtype=np.int64)
    dst = np.asarray(edge_index[1], dtype=np.int64)
    w = np.asarray(edge_weight, dtype=np.float32)

    core = dst // NC_N
    drel = dst % NC_N

    deg_full = np.zeros(N, dtype=np.float64)
    np.add.at(deg_full, dst, w.astype(np.float64))
    dinv_full = 1.0 / np.sqrt(deg_full + 2.0)

    self_drel = np.arange(NPAD, dtype=np.int64)
    self_w = np.full(NPAD, 2.0, dtype=np.float32)

    groups = [[[None, None, None] for _ in range(NBLK)] for _ in range(P)]
    for c in range(P):
        m = core == c
        s_all = np.concatenate(
            [src[m], np.minimum(self_drel, NC_N - 1) + c * NC_N]
        )
        d_all = np.concatenate([drel[m], self_drel])
        w_all = np.concatenate([w[m], self_w])
        blk = d_all // BW
        rel = (d_all % BW).astype(np.float32)
        is_local = (s_all >= c * NC_N) & (s_all < (c + 1) * NC_N)
        lo = s_all < SPLIT
        for b in range(NBLK):
            mb = blk == b
            for s, ms in (
                (0, mb & is_local),
                (1, mb & ~is_local & lo),
                (2, mb & ~is_local & ~lo),
            ):
                base = c * NC_N if s == 0 else (0 if s == 1 else SPLIT)
                idx = s_all[ms] - base
                groups[c][b][s] = (idx.astype(np.int16), rel[ms], w_all[ms])

    tcnt = np.zeros((3, NBLK), dtype=np.int64)
    for b in range(NBLK):
        for s in range(3):
            for c in range(P):
                tcnt[s, b] = max(
                    tcnt[s, b], -(-len(groups[c][b][s][0]) // 128)
                )
        tcnt[0, b] = max(tcnt[0, b], 1)

    TS = [int(tcnt[s].sum()) for s in range(3)]
    TT = sum(TS)

    per_core = []
    for c in range(P):
        idx_bufs = [np.zeros(TS[s] * 128, dtype=np.int16) for s in range(3)]
        # meta[e] = (dst_rel, w) in PE consumption order: per block, local
        # tiles then remote-lo then remote-hi; padding entries keep w=0.
        meta_rel = np.zeros(TT * 128, dtype=np.float32)
        meta_w = np.zeros(TT * 128, dtype=np.float32)
        offs = [0, 0, 0]
        om = 0
        for b in range(NBLK):
            for s in range(3):
                idx, rel, ww = groups[c][b][s]
                n = len(idx)
                cnt = int(tcnt[s, b])
                off = offs[s]
                idx_bufs[s][off * 128 : off * 128 + n] = idx
                meta_rel[om * 128 : om * 128 + n] = rel
                meta_w[om * 128 : om * 128 + n] = ww
                om += cnt
                offs[s] += cnt

        # wrapped int16 index layout: idx i lives at [i % 16, i // 16],
        # replicated 8x along partitions (one stripe per Q7 core)
        wraps = [
            np.ascontiguousarray(np.tile(ib.reshape(-1, 16).T, (8, 1)))
            for ib in idx_bufs
        ]
        # meta in partition-major tile layout: edge t*128+p -> [p, 2t + {0,1}]
        meta = np.empty((128, 2 * TT), dtype=np.float32)
        meta[:, 0::2] = meta_rel.reshape(TT, 128).T
        meta[:, 1::2] = meta_w.reshape(TT, 128).T

        dinv_c = np.zeros(NPAD, dtype=np.float32)
        dinv_c[:NC_N] = dinv_full[c * NC_N : (c + 1) * NC_N]
        dinvrep = np.ascontiguousarray(
            np.broadcast_to(dinv_c, (128, NPAD))
        ).astype(np.float32)
        dinvT = np.ascontiguousarray(dinv_c.reshape(NB_NODE, 128).T)

        per_core.append(
            {
                "lcidx": wraps[0],
                "loidx": wraps[1],
                "hiidx": wraps[2],
                "meta": meta,
                "dinvrep": dinvrep,
                "dinvT": dinvT,
            }
        )

    return tcnt, per_core


def _build_program(tcnt, single_core=False):
    # single_core=True swaps the AllGather for a local DMA copy and builds a
    # 1-device module, so the cost-model TimelineSim (single-core only) can
    # profile the kernel; numerics of remote nodes are wrong in that mode.
    TS = [int(tcnt[s].sum()) for s in range(3)]
    TT = sum(TS)
    nc = bacc.Bacc(
        "TRN2",
        target_bir_lowering=False,
        debug=False,
        enable_asserts=False,
        num_devices=1 if single_core else P,
    )

    # ---- I/O -------------------------------------------------------------
    xT_d = nc.dram_tensor("xT", [IN_D, NC_N], FP, kind="ExternalInput")
    lcidx_d = nc.dram_tensor("lcidx", [128, TS[0] * 8], I16, kind="ExternalInput")
    loidx_d = nc.dram_tensor("loidx", [128, TS[1] * 8], I16, kind="ExternalInput")
    hiidx_d = nc.dram_tensor("hiidx", [128, TS[2] * 8], I16, kind="ExternalInput")
    meta_d = nc.dram_tensor("meta", [128, 2 * TT], FP, kind="ExternalInput")
    dinvrep_d = nc.dram_tensor("dinvrep", [128, NPAD], FP, kind="ExternalInput")
    dinvT_d = nc.dram_tensor("dinvT", [128, NB_NODE], FP, kind="ExternalInput")
    w_d = {
        name: nc.dram_tensor(name, shape, FP, kind="ExternalInput")
        for name, shape in [
            ("W1", [IN_D, H]),
            ("W2", [H, H]),
            ("W3", [H, H]),
            ("Wg1", [H, H]),
            ("Wg2", [H, H]),
            ("Wg3", [H, H]),
            ("Wg4", [H, H]),
            ("Wh", [H, OUT_D]),
        ]
    }
    # bias columns: 0..2 = b1..b3, 3..6 = bg1..bg4, 7..13 = negated, 14 = bh
    bias_d = nc.dram_tensor("bias", [128, 16], FP, kind="ExternalInput")
    iota_d = nc.dram_tensor("iota256", [128, BW], BF, kind="ExternalInput")
    out_d = nc.dram_tensor("out", [OUT_D, NC_N], FP, kind="ExternalOutput")

    rg = [list(range(P))]

    with tile.TileContext(nc) as tc, ExitStack() as ctx:
        const = ctx.enter_context(tc.tile_pool(name="const", bufs=1))
        big = ctx.enter_context(tc.tile_pool(name="big", bufs=1))
        swp = ctx.enter_context(tc.tile_pool(name="swp", bufs=48))
        epp = ctx.enter_context(tc.tile_pool(name="epp", bufs=2))
        idxp = ctx.enter_context(tc.tile_pool(name="idxp", bufs=2))
        vlc_p = ctx.enter_context(tc.tile_pool(name="vlc", bufs=3))
        vlo_p = ctx.enter_context(tc.tile_pool(name="vlo", bufs=3))
        vhi_p = ctx.enter_context(tc.tile_pool(name="vhi", bufs=3))
        metap = ctx.enter_context(tc.tile_pool(name="metap", bufs=3))
        ps_dense = ctx.enter_context(tc.tile_pool(name="psd", bufs=2, space="PSUM"))
        ps_blk = ctx.enter_context(tc.tile_pool(name="psb", bufs=2, space="PSUM"))
        ps_tr = ctx.enter_context(tc.tile_pool(name="pst", bufs=2, space="PSUM"))
        dram = ctx.enter_context(tc.tile_pool(name="dram", bufs=2, space="DRAM"))

        # ---- constants ----------------------------------------------------
        def load_const(shape, src_ap, name, dtype=FP):
            t = const.tile(shape, dtype, tag=name)
            nc.sync.dma_start(t[:], src_ap)
            return t

        w_sb = {k: load_const(list(v.shape), v[:], k) for k, v in w_d.items()}
        bias = load_const([128, 16], bias_d[:], "bias")
        iota = load_const([128, BW], iota_d[:], "iota", BF)
        dinvT = load_const([128, NB_NODE], dinvT_d[:], "dinvT")

        h_sb = big.tile([128, NPAD], FP, tag="h")
        dinvrep = big.tile([128, NPAD], FP, tag="dinvrep")
        nc.sync.dma_start(dinvrep[:], dinvrep_d[:])

        # consumption-order bookkeeping
        s_of = [[], [], []]
        meta_of = []
        offs = [0, 0, 0]
        om = 0
        for b in range(NBLK):
            meta_of.append(om)
            for s in range(3):
                s_of[s].append(offs[s])
                offs[s] += int(tcnt[s, b])
                om += int(tcnt[s, b])

        n_mchunk = -(-TT // MC)

        def emit_meta_chunks():
            mts = []
            for i in range(n_mchunk):
                cols = min(MC, TT - i * MC)
                mt = metap.tile([128, 2 * MC], FP, tag="meta")
                nc.sync.dma_start(
                    mt[:, : 2 * cols], meta_d[:, 2 * i * MC : 2 * (i * MC + cols)]
                )
                mts.append(mt)
            return mts

        def sw_tile(mts, g):
            """[128 edge, BW dst] one-hot(dst_rel)*w scatter tile for
            consumption-order tile g, built on the vector engine."""
            mt = mts[g // MC]
            o = 2 * (g % MC)
            sw = swp.tile([128, BW], BF, tag="sw")
            nc.vector.tensor_scalar(
                sw[:],
                iota[:],
                mt[:, o : o + 1],
                mt[:, o + 1 : o + 2],
                ALU.is_equal,
                ALU.mult,
            )
            return sw

        # ---- embedding MLP -------------------------------------------------

        def elu_ep(dst_ap, ps_ap, bcol, cw):
            # DVE-heavy ELU: r = max(x+b, 0), m = min(x+b, 0) on DVE,
            # e = exp(m) on ACT, out = (e-1) + r on DVE.
            r = epp.tile([128, 512], FP, tag="r")
            nm = epp.tile([128, 512], FP, tag="nm")
            e2 = epp.tile([128, 512], FP, tag="e2")
            nc.vector.tensor_scalar(
                r[:, :cw], ps_ap, bias[:, bcol : bcol + 1], 0.0, ALU.add, ALU.max
            )
            nc.vector.tensor_scalar(
                nm[:, :cw], ps_ap, bias[:, bcol : bcol + 1], 0.0, ALU.add, ALU.min
            )
            nc.scalar.activation(e2[:, :cw], nm[:, :cw], ACT_F.Exp)
            nc.vector.scalar_tensor_tensor(
                dst_ap, e2[:, :cw], -1.0, r[:, :cw], ALU.add, ALU.add
            )

        for off, cw in CHUNKS:
            xc = epp.tile([IN_D, 512], FP, tag="xc")
            real = max(0, min(cw, NC_N - off))
            if real < cw:
                nc.vector.memset(xc[:, :cw], 0.0)
            if real > 0:
                nc.sync.dma_start(xc[:, :real], xT_d[:, off : off + real])
            ps = ps_dense.tile([128, 512], FP, tag="dense")
            nc.tensor.matmul(
                ps[:, :cw], w_sb["W1"][:IN_D, :], xc[:IN_D, :cw]
            )
            elu_ep(h_sb[:, off : off + cw], ps[:, :cw], 0, cw)
        for wname, bcol in [("W2", 1), ("W3", 2)]:
            for off, cw in CHUNKS:
                ps = ps_dense.tile([128, 512], FP, tag="dense")
                nc.tensor.matmul(
                    ps[:, :cw], w_sb[wname][:], h_sb[:, off : off + cw]
                )
                elu_ep(h_sb[:, off : off + cw], ps[:, :cw], bcol, cw)

        # ---- GCN layers ---------------------------------------------------
        n_chunk = [-(-TS[s] // C_TILES) for s in range(3)]

        for layer in range(4):
            wg = w_sb[f"Wg{layer + 1}"]
            bcol = 3 + layer

            # t-block (node-major) = h_blk^T @ Wg, prescale by dinv, write to
            # the AllGather input.  lhsT = h slice puts nodes on the output
            # partition axis directly, so no transposes are needed.
            agin = dram.tile([NC_N, H], BF, tag="agin")
            tfull = dram.tile([N, H], BF, tag="tfull", addr_space="Shared")
            for b in range(NB_NODE):
                rows = min(128, NC_N - b * 128)
                if rows <= 0:
                    continue
                trp = ps_tr.tile([128, 128], FP, tag="tr")
                nc.tensor.matmul(trp[:], h_sb[:, b * 128 : (b + 1) * 128], wg[:])
                tt = epp.tile([128, 128], BF, tag="tt")
                nc.scalar.activation(
                    tt[:], trp[:], ACT_F.Copy, scale=dinvT[:, b : b + 1]
                )
                nc.sync.dma_start(agin[b * 128 : b * 128 + rows, :], tt[:rows, :])

            if single_core:
                nc.sync.dma_start(tfull[:NC_N, :], agin[:])
            else:
                nc.gpsimd.collective_compute(
                    "AllGather",
                    ALU.bypass,
                    replica_groups=rg,
                    ins=[agin[:]],
                    outs=[tfull[:]],
                )

            def emit_gathers(nchunk, total_tiles, idx_dram, table_ap, pool, tag):
                chunks = []
                for i in range(nchunk):
                    nt = min(C_TILES, total_tiles - i * C_TILES)
                    it = idxp.tile([128, C_TILES * 8], I16, tag=f"i{tag}")
                    nc.sync.dma_start(
                        it[:, : nt * 8],
                        idx_dram[:, i * C_TILES * 8 : i * C_TILES * 8 + nt * 8],
                    )
                    v = pool.tile([128, C_TILES, 128], BF, tag=tag)
                    nc.gpsimd.dma_gather(
                        v[:, :nt, :], table_ap, it[:, : nt * 8],
                        nt * 128, nt * 128, H, single_packet=False,
                    )
                    chunks.append(v)
                return chunks

            vlc = emit_gathers(
                n_chunk[0], TS[0], lcidx_d, agin[:, :], vlc_p, "vlc"
            )
            vlo = emit_gathers(
                n_chunk[1], TS[1], loidx_d, tfull[:, :], vlo_p, "vlo"
            )
            vhi = emit_gathers(
                n_chunk[2], TS[2], hiidx_d, tfull[SPLIT:, :], vhi_p, "vhi"
            )
            vstreams = (vlc, vlo, vhi)
            mts = emit_meta_chunks()

            # per-block scatter-accumulate + epilogue (local tiles first:
            # they are ready before the AllGather completes)
            for b in range(NBLK):
                nt_s = [int(tcnt[s, b]) for s in range(3)]
                ntile = sum(nt_s)
                agg = ps_blk.tile([128, BW], FP, tag="agg")
                t = 0
                for s in range(3):
                    for k in range(nt_s[s]):
                        sw = sw_tile(mts, meta_of[b] + t)
                        g = s_of[s][b] + k
                        v = vstreams[s][g // C_TILES][:, g % C_TILES, :]
                        nc.tensor.matmul(
                            agg[:], v, sw[:],
                            start=(t == 0), stop=(t == ntile - 1),
                        )
                        t += 1
                vv = epp.tile([128, BW], FP, tag="vv")
                nc.vector.tensor_tensor(
                    vv[:], agg[:], dinvrep[:, b * BW : (b + 1) * BW], ALU.mult
                )
                rb = epp.tile([128, BW], FP, tag="rb")
                nmb = epp.tile([128, BW], FP, tag="nmb")
                eb = epp.tile([128, BW], FP, tag="eb")
                nc.scalar.activation(
                    rb[:], vv[:], ACT_F.Relu, bias=bias[:, bcol : bcol + 1]
                )
                nc.scalar.activation(
                    nmb[:], vv[:], ACT_F.Relu,
                    bias=bias[:, bcol + 7 : bcol + 8], scale=-1.0,
                )
                nc.scalar.activation(eb[:], nmb[:], ACT_F.Exp, scale=-1.0)
                nc.vector.scalar_tensor_tensor(
                    h_sb[:, b * BW : (b + 1) * BW],
                    eb[:], -1.0, rb[:], ALU.add, ALU.add,
                )

        # ---- head ----------------------------------------------------------
        for off, cw in CHUNKS:
            cw = min(cw, NC_N - off)
            ps = ps_dense.tile([128, 512], FP, tag="dense")
            nc.tensor.matmul(
                ps[:OUT_D, :cw], w_sb["Wh"][:], h_sb[:, off : off + cw]
            )
            oc = epp.tile([OUT_D, 512], FP, tag="outc")
            nc.scalar.activation(
                oc[:, :cw], ps[:OUT_D, :cw], ACT_F.Identity,
                bias=bias[:OUT_D, 14:15],
            )
            nc.sync.dma_start(out_d[:, off : off + cw], oc[:, :cw])

    nc.compile()
    return nc


def _make_in_maps(inputs, per_core):
    x = np.asarray(inputs["x"], dtype=np.float32)
    bias = np.zeros((128, 16), dtype=np.float32)
    for j, nm in enumerate(["b1", "b2", "b3", "bg1", "bg2", "bg3", "bg4"]):
        b = np.asarray(inputs[nm], dtype=np.float32)
        bias[:, j] = b
        bias[:, j + 7] = -b
    bias[:OUT_D, 14] = np.asarray(inputs["bh"], dtype=np.float32)

    import ml_dtypes

    shared = {
        "bias": bias,
        "iota256": np.tile(
            np.arange(BW, dtype=np.float32), (128, 1)
        ).astype(ml_dtypes.bfloat16),
    }
    for nm in ["W1", "W2", "W3", "Wg1", "Wg2", "Wg3", "Wg4", "Wh"]:
        shared[nm] = np.ascontiguousarray(np.asarray(inputs[nm], np.float32))

    in_maps = []
    for c in range(P):
        m = dict(shared)
        m["xT"] = np.ascontiguousarray(x[c * NC_N : (c + 1) * NC_N].T)
        m.update(per_core[c])
        in_maps.append(m)
    return in_maps


def run(inputs, trace=False):
    """Run the distributed kernel; returns (out [N, OUT_D] fp32, results)."""
    tcnt, per_core = _prep_edges(inputs["edge_index"], inputs["edge_weight"])
    nc = _build_program(tcnt)
    in_maps = _make_in_maps(inputs, per_core)
    res = run_bass_kernel_spmd(nc, in_maps, list(range(P)), trace=trace)
    out = np.concatenate(
        [res.results[c]["out"].T for c in range(P)], axis=0
    ).astype(np.float32)
    return out, res


def kernel(**inputs):
    out, _ = run(inputs, trace=False)
    return out



# revision 4
# speedup vs baseline: 1.3716x; 1.3716x over previous
"""GCN (4-layer, improved self-loops) on 8 Trainium2 NeuronCores.

Sharding: 1D node partition (6250 nodes/core); edges partitioned by
destination-node owner; per layer the raw features t = h@Wg are AllGathered
into a full bf16 DRAM table on every core, then each core gathers per-edge
source rows with dma_gather and scatter-adds them into per-destination-block
PSUM tiles via one-hot matmuls on the TensorEngine.

The full GCN normalization (w_e * dinv[src] * dinv[dst], and the self-loop
coefficient 2*dinv^2) is folded into the per-edge one-hot weights on the
host, so no on-chip pre/post scaling is needed:
    h_next = elu(sum_e w'_e * t[src_e] + b)
Self-loop contributions use the node-major t tiles already resident in SBUF
as scatter lhsT directly (no DMA gather, no table read).

The t table uses a permuted row layout (row = (m%128)*NB + m//128 for local
node m) so the whole per-layer table emit is one flat SBUF->DRAM copy of the
node-major tile; gather indices bake the permutation in on the host.  Gather
indices are int16, so the gathered table is split at row 32768 into lo/hi
streams.  Everything on the PE runs bf16 (1 cycle/row); destination blocks
are 128 wide to halve PE/DVE cost per edge vs 256-wide blocks.
"""

import numpy as np
from contextlib import ExitStack

try:
    import concourse.bass as bass
except ImportError:  # pragma: no cover
    import sys

    sys.path.insert(0, "/opt/trn_rl_repo")
    import concourse.bass as bass

import concourse.bacc as bacc
import concourse.mybir as mybir
import concourse.tile as tile
from concourse.bass_utils import run_bass_kernel_spmd

FP = mybir.dt.float32
BF = mybir.dt.bfloat16
I16 = mybir.dt.int16

N = 50000
E = 800000
IN_D = 64
H = 128
OUT_D = 16
P = 8
NC_N = N // P            # 6250 nodes per core
BW = 128                 # destination-block width (scatter matmul moving dim)
NBLK = -(-NC_N // BW)    # 49 destination blocks per core
NPAD = NBLK * BW         # 6272
SPLIT = 32768            # lo/hi split of permuted tfull rows (int16 indices)
C_TILES = 32             # 128-edge tiles per dma_gather call

# dense-matmul column chunks over the padded node dim
CHUNKS = [(k * 512, 512) for k in range(12)] + [(6144, 128)]

ALU = mybir.AluOpType
ACT_F = mybir.ActivationFunctionType


def _rowperm_local(m):
    """Permuted row index of local node m in the [NPAD, H] table (the flat
    view of the node-major [128, NBLK*H] SBUF tile)."""
    return (m % BW) * NBLK + m // BW


def _prep_edges(edge_index, edge_weight):
    """Host preprocessing: partition edges by dst owner, fold the full GCN
    normalization into per-edge weights, split local/remote-lo/remote-hi by
    source table row, group by BW-dst block, pad each (core, block, stream)
    group to a common (max-over-cores) tile count.

    Returns (tcnt, per_core); per_core[c] has lcidx/loidx/hiidx/meta arrays.
    """
    import ml_dtypes

    src = np.asarray(edge_index[0], dtype=np.int64)
    dst = np.asarray(edge_index[1], dtype=np.int64)
    w = np.asarray(edge_weight, dtype=np.float32)

    core = dst // NC_N
    drel = dst % NC_N

    deg_full = np.zeros(N, dtype=np.float64)
    np.add.at(deg_full, dst, w.astype(np.float64))
    dinv_full = (1.0 / np.sqrt(deg_full + 2.0)).astype(np.float32)

    wn = w * dinv_full[src] * dinv_full[dst]   # folded edge norm

    # permuted global table row for source node s
    src_core = src // NC_N
    src_m = src % NC_N
    row_global = src_core * NPAD + (src_m % BW) * NBLK + src_m // BW
    row_local = (src_m % BW) * NBLK + src_m // BW

    groups = [[[None] * 3 for _ in range(NBLK)] for _ in range(P)]
    for c in range(P):
        mask = core == c
        s_core, d_all, w_all = src_core[mask], drel[mask], wn[mask]
        rg, rl = row_global[mask], row_local[mask]
        blk = d_all // BW
        rel = (d_all % BW).astype(np.float32)
        is_local = s_core == c
        lo = rg < SPLIT
        for b in range(NBLK):
            mb = blk == b
            for s, ms, base in (
                (0, mb & is_local, None),
                (1, mb & ~is_local & lo, 0),
                (2, mb & ~is_local & ~lo, SPLIT),
            ):
                idx = rl[ms] if s == 0 else rg[ms] - base
                # sort by source row for DRAM locality
                o = np.argsort(idx, kind="stable")
                groups[c][b][s] = (
                    idx[o].astype(np.int16), rel[ms][o], w_all[ms][o],
                )

    tcnt = np.zeros((3, NBLK), dtype=np.int64)
    for b in range(NBLK):
        for s in range(3):
            for c in range(P):
                tcnt[s, b] = max(tcnt[s, b], -(-len(groups[c][b][s][0]) // 128))

    TS = [int(tcnt[s].sum()) for s in range(3)]
    TT = NBLK + sum(TS)      # +1 self tile per block

    per_core = []
    for c in range(P):
        dinv_c = np.zeros(NPAD, dtype=np.float32)
        dinv_c[:NC_N] = dinv_full[c * NC_N : (c + 1) * NC_N]
        selfw = 2.0 * dinv_c * dinv_c

        idx_bufs = [np.zeros(TS[s] * 128, dtype=np.int16) for s in range(3)]
        # meta[e] = (dst_rel, w) in PE consumption order: per block, the self
        # tile first, then local/remote-lo/remote-hi; padding keeps w=0.
        meta_rel = np.zeros(TT * 128, dtype=np.float32)
        meta_w = np.zeros(TT * 128, dtype=np.float32)
        offs = [0, 0, 0]
        om = 0
        iota128 = np.arange(128, dtype=np.float32)
        for b in range(NBLK):
            meta_rel[om * 128 : om * 128 + 128] = iota128
            meta_w[om * 128 : om * 128 + 128] = selfw[b * 128 : (b + 1) * 128]
            om += 1
            for s in range(3):
                idx, rel, ww = groups[c][b][s]
                n = len(idx)
                cnt = int(tcnt[s, b])
                off = offs[s]
                idx_bufs[s][off * 128 : off * 128 + n] = idx
                meta_rel[om * 128 : om * 128 + n] = rel
                meta_w[om * 128 : om * 128 + n] = ww
                om += cnt
                offs[s] += cnt

        # wrapped int16 index layout: idx i lives at [i % 16, i // 16],
        # replicated 8x along partitions (one stripe per Q7 core)
        wraps = [
            np.ascontiguousarray(np.tile(ib.reshape(-1, 16).T, (8, 1)))
            if len(ib)
            else np.zeros((128, 0), dtype=np.int16)
            for ib in idx_bufs
        ]
        # meta in partition-major tile layout: edge t*128+p -> [p, 2t + {0,1}]
        meta = np.empty((128, 2 * TT), dtype=np.float32)
        meta[:, 0::2] = meta_rel.reshape(TT, 128).T
        meta[:, 1::2] = meta_w.reshape(TT, 128).T

        per_core.append(
            {
                "lcidx": wraps[0],
                "loidx": wraps[1],
                "hiidx": wraps[2],
                "meta": meta,
            }
        )

    return tcnt, per_core


def _build_program(tcnt, single_core=False):
    # single_core=True swaps the AllGather for a local DMA copy and builds a
    # 1-device module, so the cost-model TimelineSim (single-core only) can
    # profile the kernel; numerics of remote nodes are wrong in that mode.
    TS = [int(tcnt[s].sum()) for s in range(3)]
    TT = NBLK + sum(TS)
    nc = bacc.Bacc(
        "TRN2",
        target_bir_lowering=False,
        debug=False,
        enable_asserts=False,
        num_devices=1 if single_core else P,
    )

    # ---- I/O -------------------------------------------------------------
    xT_d = nc.dram_tensor("xT", [IN_D, NC_N], BF, kind="ExternalInput")
    lcidx_d = nc.dram_tensor("lcidx", [128, max(TS[0], 1) * 8], I16, kind="ExternalInput")
    loidx_d = nc.dram_tensor("loidx", [128, max(TS[1], 1) * 8], I16, kind="ExternalInput")
    hiidx_d = nc.dram_tensor("hiidx", [128, max(TS[2], 1) * 8], I16, kind="ExternalInput")
    meta_d = nc.dram_tensor("meta", [128, 2 * TT], FP, kind="ExternalInput")
    w_d = {
        name: nc.dram_tensor(name, shape, BF, kind="ExternalInput")
        for name, shape in [
            ("W1", [IN_D, H]),
            ("W2", [H, H]),
            ("W3", [H, H]),
            ("Wg1", [H, H]),
            ("Wg2", [H, H]),
            ("Wg3", [H, H]),
            ("Wg4", [H, H]),
            ("Wh", [H, OUT_D]),
        ]
    }
    # bias columns: 0..2 = b1..b3, 3..6 = bg1..bg4, 7..13 = negated, 14 = bh
    bias_d = nc.dram_tensor("bias", [128, 16], FP, kind="ExternalInput")
    iota_d = nc.dram_tensor("iota128", [128, BW], BF, kind="ExternalInput")
    out_d = nc.dram_tensor("out", [OUT_D, NC_N], FP, kind="ExternalOutput")

    rg = [list(range(P))]

    with tile.TileContext(nc) as tc, ExitStack() as ctx:
        const = ctx.enter_context(tc.tile_pool(name="const", bufs=1))
        big = ctx.enter_context(tc.tile_pool(name="big", bufs=1))
        swp = ctx.enter_context(tc.tile_pool(name="swp", bufs=48))
        epp = ctx.enter_context(tc.tile_pool(name="epp", bufs=3))
        vlc_p = ctx.enter_context(tc.tile_pool(name="vlc", bufs=3))
        vlo_p = ctx.enter_context(tc.tile_pool(name="vlo", bufs=3))
        vhi_p = ctx.enter_context(tc.tile_pool(name="vhi", bufs=3))
        ps_dense = ctx.enter_context(tc.tile_pool(name="psd", bufs=2, space="PSUM"))
        ps_blk = ctx.enter_context(tc.tile_pool(name="psb", bufs=3, space="PSUM"))
        ps_tr = ctx.enter_context(tc.tile_pool(name="pst", bufs=2, space="PSUM"))
        dram = ctx.enter_context(tc.tile_pool(name="dram", bufs=2, space="DRAM"))

        # ---- constants ----------------------------------------------------
        def load_const(shape, src_ap, name, dtype=FP):
            t = const.tile(shape, dtype, tag=name)
            nc.sync.dma_start(t[:], src_ap)
            return t

        w_sb = {k: load_const(list(v.shape), v[:], k, BF) for k, v in w_d.items()}
        bias = load_const([128, 16], bias_d[:], "bias")
        iota = load_const([128, BW], iota_d[:], "iota", BF)
        meta_sb = load_const([128, 2 * TT], meta_d[:], "meta")
        idx_sb = [
            load_const([128, max(TS[s], 1) * 8], d[:], f"idx{s}", I16)
            for s, d in enumerate((lcidx_d, loidx_d, hiidx_d))
        ]

        h_sb = big.tile([128, NPAD], BF, tag="h")
        tt_sb = big.tile([128, NPAD], BF, tag="tt")
        xc = big.tile([IN_D, NPAD], BF, tag="xc")
        oc = big.tile([OUT_D, NPAD], FP, tag="oc")

        nc.vector.memset(xc[:, NC_N:], 0.0)
        nc.sync.dma_start(xc[:, :NC_N], xT_d[:])

        # consumption-order bookkeeping
        s_of = [[], [], []]     # stream tile offset per block
        meta_of = []            # meta tile index of block's first (self) tile
        offs = [0, 0, 0]
        om = 0
        for b in range(NBLK):
            meta_of.append(om)
            om += 1
            for s in range(3):
                s_of[s].append(offs[s])
                offs[s] += int(tcnt[s, b])
                om += int(tcnt[s, b])

        def sw_tile(g):
            """[128 edge, BW dst] one-hot(dst_rel)*w scatter tile for
            consumption-order tile g, built on the vector engine."""
            sw = swp.tile([128, BW], BF, tag="sw")
            nc.vector.tensor_scalar(
                sw[:],
                iota[:],
                meta_sb[:, 2 * g : 2 * g + 1],
                meta_sb[:, 2 * g + 1 : 2 * g + 2],
                ALU.is_equal,
                ALU.mult,
            )
            return sw

        # ---- embedding MLP -------------------------------------------------

        def elu_ep(dst_ap, ps_ap, bcol, cw):
            # r = max(x+b, 0) on DVE; nm = relu(-(x+b)), e = exp(-nm) on ACT;
            # out = (e-1) + r on DVE.
            r = epp.tile([128, 512], FP, tag="r")
            nm = epp.tile([128, 512], FP, tag="nm")
            e2 = epp.tile([128, 512], FP, tag="e2")
            nc.vector.tensor_scalar(
                r[:, :cw], ps_ap, bias[:, bcol : bcol + 1], 0.0, ALU.add, ALU.max
            )
            nc.scalar.activation(
                nm[:, :cw], ps_ap, ACT_F.Relu,
                bias=bias[:, bcol + 7 : bcol + 8], scale=-1.0,
            )
            nc.scalar.activation(e2[:, :cw], nm[:, :cw], ACT_F.Exp, scale=-1.0)
            nc.vector.scalar_tensor_tensor(
                dst_ap, e2[:, :cw], -1.0, r[:, :cw], ALU.add, ALU.add
            )

        for off, cw in CHUNKS:
            ps = ps_dense.tile([128, 512], FP, tag="dense")
            nc.tensor.matmul(
                ps[:, :cw], w_sb["W1"][:IN_D, :], xc[:IN_D, off : off + cw]
            )
            elu_ep(h_sb[:, off : off + cw], ps[:, :cw], 0, cw)
        for wname, bcol in [("W2", 1), ("W3", 2)]:
            for off, cw in CHUNKS:
                ps = ps_dense.tile([128, 512], FP, tag="dense")
                nc.tensor.matmul(
                    ps[:, :cw], w_sb[wname][:], h_sb[:, off : off + cw]
                )
                elu_ep(h_sb[:, off : off + cw], ps[:, :cw], bcol, cw)

        # ---- GCN layers ---------------------------------------------------
        n_chunk = [-(-TS[s] // C_TILES) for s in range(3)]
        # emit gathers interleaved by the first block that consumes each chunk
        blk_of_tile = [
            np.searchsorted(
                np.array(s_of[s]), np.arange(max(TS[s], 1)), side="right"
            )
            - 1
            for s in range(3)
        ]
        chunk_order = sorted(
            [(int(blk_of_tile[s][c * C_TILES]), s, c)
             for s in range(3) for c in range(n_chunk[s])]
        )

        for layer in range(4):
            wg = w_sb[f"Wg{layer + 1}"]
            bcol = 3 + layer

            # t (node-major) = h_blk^T @ Wg per block; lhsT = h slice puts
            # nodes on the output partition axis, so no transposes needed.
            for b in range(NBLK):
                trp = ps_tr.tile([128, 128], FP, tag="tr")
                nc.tensor.matmul(trp[:], h_sb[:, b * BW : (b + 1) * BW], wg[:])
                nc.scalar.activation(
                    tt_sb[:, b * BW : (b + 1) * BW], trp[:], ACT_F.Copy
                )

            # one flat table emit; rows are host-side permuted
            agin = dram.tile([NPAD, H], BF, tag="agin")
            tfull = dram.tile([P * NPAD, H], BF, tag="tfull", addr_space="Shared")
            nc.sync.dma_start(agin[:, :], tt_sb[:])

            if single_core:
                nc.sync.dma_start(tfull[:NPAD, :], agin[:, :])
            else:
                nc.gpsimd.collective_compute(
                    "AllGather",
                    ALU.bypass,
                    replica_groups=rg,
                    ins=[agin[:]],
                    outs=[tfull[:]],
                )

            tables = (agin[:, :], tfull[:, :], tfull[SPLIT:, :])
            vpools = (vlc_p, vlo_p, vhi_p)
            vchunks = [[None] * n_chunk[s] for s in range(3)]
            for _, s, ci in chunk_order:
                nt = min(C_TILES, TS[s] - ci * C_TILES)
                v = vpools[s].tile([128, C_TILES, 128], BF, tag=f"v{s}")
                nc.gpsimd.dma_gather(
                    v[:, :nt, :], tables[s],
                    idx_sb[s][:, ci * C_TILES * 8 : (ci * C_TILES + nt) * 8],
                    nt * 128, nt * 128, H, single_packet=False,
                )
                vchunks[s][ci] = v

            # per-block scatter-accumulate + epilogue; the self tile (SBUF
            # node-major t) leads each block's accumulation group.
            for b in range(NBLK):
                nt_s = [int(tcnt[s, b]) for s in range(3)]
                ntile = 1 + sum(nt_s)
                agg = ps_blk.tile([128, BW], FP, tag="agg")
                sw = sw_tile(meta_of[b])
                nc.tensor.matmul(
                    agg[:], tt_sb[:, b * BW : (b + 1) * BW], sw[:],
                    start=True, stop=(ntile == 1),
                )
                t = 1
                for s in range(3):
                    for k in range(nt_s[s]):
                        sw = sw_tile(meta_of[b] + t)
                        g = s_of[s][b] + k
                        v = vchunks[s][g // C_TILES][:, g % C_TILES, :]
                        nc.tensor.matmul(
                            agg[:], v, sw[:],
                            start=False, stop=(t == ntile - 1),
                        )
                        t += 1
                rb = epp.tile([128, BW], BF, tag="rb")
                nmb = epp.tile([128, BW], FP, tag="nmb")
                eb = epp.tile([128, BW], BF, tag="eb")
                nc.scalar.activation(
                    rb[:], agg[:], ACT_F.Relu, bias=bias[:, bcol : bcol + 1]
                )
                nc.scalar.activation(
                    nmb[:], agg[:], ACT_F.Relu,
                    bias=bias[:, bcol + 7 : bcol + 8], scale=-1.0,
                )
                nc.scalar.activation(eb[:], nmb[:], ACT_F.Exp, scale=-1.0)
                nc.vector.scalar_tensor_tensor(
                    h_sb[:, b * BW : (b + 1) * BW],
                    eb[:], -1.0, rb[:], ALU.add, ALU.add,
                )

        # ---- head ----------------------------------------------------------
        for off, cw in CHUNKS:
            cw = min(cw, NC_N - off)
            ps = ps_dense.tile([128, 512], FP, tag="dense")
            nc.tensor.matmul(
                ps[:OUT_D, :cw], w_sb["Wh"][:], h_sb[:, off : off + cw]
            )
            nc.scalar.activation(
                oc[:, off : off + cw], ps[:OUT_D, :cw], ACT_F.Identity,
                bias=bias[:OUT_D, 14:15],
            )
        nc.sync.dma_start(out_d[:, :], oc[:, :NC_N])

    nc.compile()
    return nc


def _make_in_maps(inputs, per_core):
    import ml_dtypes

    x = np.asarray(inputs["x"], dtype=np.float32)
    bias = np.zeros((128, 16), dtype=np.float32)
    for j, nm in enumerate(["b1", "b2", "b3", "bg1", "bg2", "bg3", "bg4"]):
        b = np.asarray(inputs[nm], dtype=np.float32)
        bias[:, j] = b
        bias[:, j + 7] = -b
    bias[:OUT_D, 14] = np.asarray(inputs["bh"], dtype=np.float32)

    shared = {
        "bias": bias,
        "iota128": np.tile(
            np.arange(BW, dtype=np.float32), (128, 1)
        ).astype(ml_dtypes.bfloat16),
    }
    for nm in ["W1", "W2", "W3", "Wg1", "Wg2", "Wg3", "Wg4", "Wh"]:
        shared[nm] = np.ascontiguousarray(
            np.asarray(inputs[nm], np.float32)
        ).astype(ml_dtypes.bfloat16)

    in_maps = []
    for c in range(P):
        m = dict(shared)
        m["xT"] = np.ascontiguousarray(
            x[c * NC_N : (c + 1) * NC_N].T
        ).astype(ml_dtypes.bfloat16)
        m.update(per_core[c])
        in_maps.append(m)
    return in_maps


def run(inputs, trace=False):
    """Run the distributed kernel; returns (out [N, OUT_D] fp32, results)."""
    tcnt, per_core = _prep_edges(inputs["edge_index"], inputs["edge_weight"])
    nc = _build_program(tcnt)
    in_maps = _make_in_maps(inputs, per_core)
    res = run_bass_kernel_spmd(nc, in_maps, list(range(P)), trace=trace)
    out = np.concatenate(
        [res.results[c]["out"].T for c in range(P)], axis=0
    ).astype(np.float32)
    return out, res


def kernel(**inputs):
    out, _ = run(inputs, trace=False)
    return out


# revision 23
# speedup vs baseline: 1.4534x; 1.0596x over previous
"""GCN (4-layer, improved self-loops) on 8 Trainium2 NeuronCores.

Sharding: 1D node partition (6250 nodes/core); edges partitioned by
destination-node owner; per layer the raw features t = h@Wg are AllGathered
into a full bf16 DRAM table on every core, then each core gathers per-edge
source rows with dma_gather and scatter-adds them into per-destination-block
PSUM tiles via one-hot matmuls on the TensorEngine.

The full GCN normalization (w_e * dinv[src] * dinv[dst], and the self-loop
coefficient 2*dinv^2) is folded into the per-edge one-hot weights on the
host, so no on-chip pre/post scaling is needed:
    h_next = elu(sum_e w'_e * t[src_e] + b)
Self-loop contributions use the node-major t tiles already resident in SBUF
as scatter lhsT directly (no DMA gather, no table read).

The t table uses a permuted row layout (row = (m%128)*NB + m//128 for local
node m) so the whole per-layer table emit is one flat SBUF->DRAM copy of the
node-major tile; gather indices bake the permutation in on the host.  Gather
indices are int16, so the gathered table is split at row 32768 into lo/hi
streams.  Everything on the PE runs bf16 (1 cycle/row); destination blocks
are 128 wide to halve PE/DVE cost per edge vs 256-wide blocks.
"""

import numpy as np
from contextlib import ExitStack

try:
    import concourse.bass as bass
except ImportError:  # pragma: no cover
    import sys

    sys.path.insert(0, "/opt/trn_rl_repo")
    import concourse.bass as bass

import concourse.bacc as bacc
import concourse.mybir as mybir
import concourse.tile as tile
from concourse.bass_utils import run_bass_kernel_spmd

FP = mybir.dt.float32
BF = mybir.dt.bfloat16
I16 = mybir.dt.int16

N = 50000
E = 800000
IN_D = 64
H = 128
OUT_D = 16
P = 8
NC_N = N // P            # 6250 nodes per core
BW = 128                 # destination-block width (scatter matmul moving dim)
NBLK = -(-NC_N // BW)    # 49 destination blocks per core
NPAD = NBLK * BW         # 6272
SPLIT = 32768            # lo/hi split of permuted tfull rows (int16 indices)
C_TILES = 32             # 128-edge tiles per dma_gather call

# dense-matmul column chunks over the padded node dim
CHUNKS = [(k * 512, 512) for k in range(12)] + [(6144, 128)]

ALU = mybir.AluOpType
ACT_F = mybir.ActivationFunctionType


def _rowperm_local(m):
    """Permuted row index of local node m in the [NPAD, H] table (the flat
    view of the node-major [128, NBLK*H] SBUF tile)."""
    return (m % BW) * NBLK + m // BW


def _layout(nmax):
    """Dense per-stream slot layout shared by host prep and program build.

    Each (block, stream) group occupies slots [O[s][b], O[s][b]+nmax[s,b])
    of its stream — no tile alignment, so gather descriptors cover only the
    max-over-cores real edge count.  A 128-slot tile straddling a block
    boundary is consumed by both blocks (with w=0 masks from meta).

    Returns (O, ST, NT, uses, meta_of, TT): slot offsets, stream slot
    totals, stream tile counts, per-block list of (stream, tile) uses, the
    meta tile index of each block's self tile, and the total meta tiles.
    """
    O = [np.concatenate([[0], np.cumsum(nmax[s])]) for s in range(3)]
    ST = [int(O[s][-1]) for s in range(3)]
    NT = [-(-ST[s] // 128) for s in range(3)]
    uses = []
    meta_of = []
    om = 0
    for b in range(NBLK):
        meta_of.append(om)
        om += 1
        ub = []
        for s in range(3):
            n = int(nmax[s][b])
            if n == 0:
                continue
            t0 = int(O[s][b]) // 128
            t1 = int(O[s][b] + n - 1) // 128
            ub.extend((s, t) for t in range(t0, t1 + 1))
        om += len(ub)
        uses.append(ub)
    return O, ST, NT, uses, meta_of, om


def _prep_edges(edge_index, edge_weight):
    """Host preprocessing: partition edges by dst owner, fold the full GCN
    normalization into per-edge weights, split local/remote-lo/remote-hi by
    source table row, group by BW-dst block, pad each (core, block, stream)
    group to a common (max-over-cores) tile count.

    Returns (tcnt, per_core); per_core[c] has lcidx/loidx/hiidx/meta arrays.
    """
    import ml_dtypes

    src = np.asarray(edge_index[0], dtype=np.int64)
    dst = np.asarray(edge_index[1], dtype=np.int64)
    w = np.asarray(edge_weight, dtype=np.float32)

    core = dst // NC_N
    drel = dst % NC_N

    deg_full = np.zeros(N, dtype=np.float64)
    np.add.at(deg_full, dst, w.astype(np.float64))
    dinv_full = (1.0 / np.sqrt(deg_full + 2.0)).astype(np.float32)

    wn = w * dinv_full[src] * dinv_full[dst]   # folded edge norm

    # permuted global table row for source node s
    src_core = src // NC_N
    src_m = src % NC_N
    row_global = src_core * NPAD + (src_m % BW) * NBLK + src_m // BW
    row_local = (src_m % BW) * NBLK + src_m // BW

    groups = [[[None] * 3 for _ in range(NBLK)] for _ in range(P)]
    for c in range(P):
        mask = core == c
        s_core, d_all, w_all = src_core[mask], drel[mask], wn[mask]
        rg, rl = row_global[mask], row_local[mask]
        blk = d_all // BW
        rel = (d_all % BW).astype(np.float32)
        is_local = s_core == c
        even = rg % 2 == 0
        for b in range(NBLK):
            mb = blk == b
            # remote edges split by table-row parity: parity is uniform
            # across cores, so the max-over-cores stream sizes stay balanced
            # (an absolute row split would skew per core); idx = row//2 fits
            # int16 since the table is viewed as [rows/2, 2H] row pairs.
            for s, ms in (
                (0, mb & is_local),
                (1, mb & ~is_local & even),
                (2, mb & ~is_local & ~even),
            ):
                idx = rl[ms] if s == 0 else rg[ms] // 2
                # sort by source row for DRAM locality
                o = np.argsort(idx, kind="stable")
                groups[c][b][s] = (
                    idx[o].astype(np.int16), rel[ms][o], w_all[ms][o],
                )

    nmax = np.zeros((3, NBLK), dtype=np.int64)
    for b in range(NBLK):
        for s in range(3):
            for c in range(P):
                nmax[s, b] = max(nmax[s, b], len(groups[c][b][s][0]))

    O, ST, NT, uses, meta_of, TT = _layout(nmax)

    per_core = []
    for c in range(P):
        dinv_c = np.zeros(NPAD, dtype=np.float32)
        dinv_c[:NC_N] = dinv_full[c * NC_N : (c + 1) * NC_N]
        selfw = 2.0 * dinv_c * dinv_c

        idx_bufs = [np.zeros(NT[s] * 128, dtype=np.int16) for s in range(3)]
        # meta[e] = (dst_rel, w) per consumption-order tile USE: per block,
        # the self tile first, then each stream tile overlapping the block's
        # dense slot range; out-of-range rows keep w=0.
        meta_rel = np.zeros(TT * 128, dtype=np.float32)
        meta_w = np.zeros(TT * 128, dtype=np.float32)
        om = 0
        iota128 = np.arange(128, dtype=np.float32)
        for b in range(NBLK):
            meta_rel[om * 128 : om * 128 + 128] = iota128
            meta_w[om * 128 : om * 128 + 128] = selfw[b * 128 : (b + 1) * 128]
            om += 1
            for s, T in uses[b]:
                idx, rel, ww = groups[c][b][s]
                n = len(idx)
                o_sb = int(O[s][b])
                if T == o_sb // 128:
                    idx_bufs[s][o_sb : o_sb + n] = idx
                lo_s = max(o_sb, 128 * T)
                hi_s = min(o_sb + n, 128 * (T + 1))
                if hi_s > lo_s:
                    mo = om * 128 + (lo_s - 128 * T)
                    meta_rel[mo : mo + hi_s - lo_s] = rel[lo_s - o_sb : hi_s - o_sb]
                    meta_w[mo : mo + hi_s - lo_s] = ww[lo_s - o_sb : hi_s - o_sb]
                om += 1

        # wrapped int16 index layout: idx i lives at [i % 16, i // 16],
        # replicated 8x along partitions (one stripe per Q7 core)
        wraps = [
            np.ascontiguousarray(np.tile(ib.reshape(-1, 16).T, (8, 1)))
            if len(ib)
            else np.zeros((128, 0), dtype=np.int16)
            for ib in idx_bufs
        ]
        # meta in partition-major tile layout: edge t*128+p -> [p, 2t + {0,1}]
        meta = np.empty((128, 2 * TT), dtype=np.float32)
        meta[:, 0::2] = meta_rel.reshape(TT, 128).T
        meta[:, 1::2] = meta_w.reshape(TT, 128).T

        per_core.append(
            {
                "lcidx": wraps[0],
                "loidx": wraps[1],
                "hiidx": wraps[2],
                "meta": meta,
            }
        )

    return nmax, per_core


def _build_program(tcnt, single_core=False):
    # single_core=True swaps the AllGather for a local DMA copy and builds a
    # 1-device module, so the cost-model TimelineSim (single-core only) can
    # profile the kernel; numerics of remote nodes are wrong in that mode.
    nmax = tcnt
    O, ST, NT, uses, meta_of, TT = _layout(nmax)
    TS = NT
    nc = bacc.Bacc(
        "TRN2",
        target_bir_lowering=False,
        debug=False,
        enable_asserts=False,
        num_devices=1 if single_core else P,
    )

    # ---- I/O -------------------------------------------------------------
    xT_d = nc.dram_tensor("xT", [IN_D, NC_N], BF, kind="ExternalInput")
    lcidx_d = nc.dram_tensor("lcidx", [128, max(TS[0], 1) * 8], I16, kind="ExternalInput")
    loidx_d = nc.dram_tensor("loidx", [128, max(TS[1], 1) * 8], I16, kind="ExternalInput")
    hiidx_d = nc.dram_tensor("hiidx", [128, max(TS[2], 1) * 8], I16, kind="ExternalInput")
    meta_d = nc.dram_tensor("meta", [128, 2 * TT], FP, kind="ExternalInput")
    w_d = {
        name: nc.dram_tensor(name, shape, BF, kind="ExternalInput")
        for name, shape in [
            ("W1", [IN_D, H]),
            ("W2", [H, H]),
            ("W3", [H, H]),
            ("Wg1", [H, H]),
            ("Wg2", [H, H]),
            ("Wg3", [H, H]),
            ("Wg4", [H, H]),
            ("Wh", [H, OUT_D]),
        ]
    }
    # bias columns: 0..2 = b1..b3, 3..6 = bg1..bg4, 7..13 = negated, 14 = bh
    bias_d = nc.dram_tensor("bias", [128, 16], FP, kind="ExternalInput")
    iota_d = nc.dram_tensor("iota128", [128, BW], BF, kind="ExternalInput")
    out_d = nc.dram_tensor("out", [OUT_D, NC_N], FP, kind="ExternalOutput")

    rg = [list(range(P))]

    with tile.TileContext(nc) as tc, ExitStack() as ctx:
        const = ctx.enter_context(tc.tile_pool(name="const", bufs=1))
        big = ctx.enter_context(tc.tile_pool(name="big", bufs=1))
        swp = ctx.enter_context(tc.tile_pool(name="swp", bufs=48))
        epp = ctx.enter_context(tc.tile_pool(name="epp", bufs=3))
        vlc_p = ctx.enter_context(tc.tile_pool(name="vlc", bufs=3))
        vlo_p = ctx.enter_context(tc.tile_pool(name="vlo", bufs=3))
        vhi_p = ctx.enter_context(tc.tile_pool(name="vhi", bufs=3))
        ps_dense = ctx.enter_context(tc.tile_pool(name="psd", bufs=2, space="PSUM"))
        ps_blk = ctx.enter_context(tc.tile_pool(name="psb", bufs=3, space="PSUM"))
        ps_tr = ctx.enter_context(tc.tile_pool(name="pst", bufs=2, space="PSUM"))
        dram = ctx.enter_context(tc.tile_pool(name="dram", bufs=2, space="DRAM"))

        # ---- constants ----------------------------------------------------
        def load_const(shape, src_ap, name, dtype=FP):
            t = const.tile(shape, dtype, tag=name)
            nc.sync.dma_start(t[:], src_ap)
            return t

        w_sb = {k: load_const(list(v.shape), v[:], k, BF) for k, v in w_d.items()}
        bias = load_const([128, 16], bias_d[:], "bias")
        iota = load_const([128, BW], iota_d[:], "iota", BF)
        meta_sb = load_const([128, 2 * TT], meta_d[:], "meta")
        idx_sb = [
            load_const([128, max(TS[s], 1) * 8], d[:], f"idx{s}", I16)
            for s, d in enumerate((lcidx_d, loidx_d, hiidx_d))
        ]

        h_sb = big.tile([128, NPAD], BF, tag="h")
        tt_sb = big.tile([128, NPAD], BF, tag="tt")
        xc = big.tile([IN_D, NPAD], BF, tag="xc")
        oc = big.tile([OUT_D, NPAD], FP, tag="oc")

        nc.vector.memset(xc[:, NC_N:], 0.0)
        nc.sync.dma_start(xc[:, :NC_N], xT_d[:])

        def sw_tile(g):
            """[128 edge, BW dst] one-hot(dst_rel)*w scatter tile for
            consumption-order tile g, built on the vector engine."""
            sw = swp.tile([128, BW], BF, tag="sw")
            nc.vector.tensor_scalar(
                sw[:],
                iota[:],
                meta_sb[:, 2 * g : 2 * g + 1],
                meta_sb[:, 2 * g + 1 : 2 * g + 2],
                ALU.is_equal,
                ALU.mult,
            )
            return sw

        # ---- embedding MLP -------------------------------------------------

        def elu_ep(dst_ap, ps_ap, bcol, cw):
            # r = max(x+b, 0) on DVE; nm = relu(-(x+b)), e = exp(-nm) on ACT;
            # out = (e-1) + r on DVE.
            r = epp.tile([128, 512], FP, tag="r")
            nm = epp.tile([128, 512], FP, tag="nm")
            e2 = epp.tile([128, 512], FP, tag="e2")
            nc.vector.tensor_scalar(
                r[:, :cw], ps_ap, bias[:, bcol : bcol + 1], 0.0, ALU.add, ALU.max
            )
            nc.scalar.activation(
                nm[:, :cw], ps_ap, ACT_F.Relu,
                bias=bias[:, bcol + 7 : bcol + 8], scale=-1.0,
            )
            nc.scalar.activation(e2[:, :cw], nm[:, :cw], ACT_F.Exp, scale=-1.0)
            nc.vector.scalar_tensor_tensor(
                dst_ap, e2[:, :cw], -1.0, r[:, :cw], ALU.add, ALU.add
            )

        for off, cw in CHUNKS:
            ps = ps_dense.tile([128, 512], FP, tag="dense")
            nc.tensor.matmul(
                ps[:, :cw], w_sb["W1"][:IN_D, :], xc[:IN_D, off : off + cw]
            )
            elu_ep(h_sb[:, off : off + cw], ps[:, :cw], 0, cw)
        for wname, bcol in [("W2", 1), ("W3", 2)]:
            for off, cw in CHUNKS:
                ps = ps_dense.tile([128, 512], FP, tag="dense")
                nc.tensor.matmul(
                    ps[:, :cw], w_sb[wname][:], h_sb[:, off : off + cw]
                )
                elu_ep(h_sb[:, off : off + cw], ps[:, :cw], bcol, cw)

        # ---- GCN layers ---------------------------------------------------
        n_chunk = [-(-NT[s] // C_TILES) for s in range(3)]
        # emit gather chunks interleaved by first consuming block
        chunk_order = sorted(
            (max(0, int(np.searchsorted(O[s], ci * C_TILES * 128, "right")) - 1),
             s, ci)
            for s in range(3) for ci in range(n_chunk[s])
        )

        for layer in range(4):
            wg = w_sb[f"Wg{layer + 1}"]
            bcol = 3 + layer

            # t (node-major) = h_blk^T @ Wg per block; lhsT = h slice puts
            # nodes on the output partition axis, so no transposes needed.
            for b in range(NBLK):
                trp = ps_tr.tile([128, 128], FP, tag="tr")
                nc.tensor.matmul(trp[:], h_sb[:, b * BW : (b + 1) * BW], wg[:])
                nc.scalar.activation(
                    tt_sb[:, b * BW : (b + 1) * BW], trp[:], ACT_F.Copy
                )

            # one flat table emit; rows are host-side permuted.  tfull is
            # viewed as [row-pairs, 2H] so the even/odd gather streams can
            # address all P*NPAD rows with int16 pair indices + elem_step.
            agin = dram.tile([NPAD, H], BF, tag="agin")
            tfull = dram.tile(
                [P * NPAD // 2, 2 * H], BF, tag="tfull", addr_space="Shared"
            )
            nc.sync.dma_start(agin[:, :], tt_sb[:])

            if single_core:
                nc.sync.dma_start(tfull[: NPAD // 2, :], agin[:, :])
            else:
                nc.gpsimd.collective_compute(
                    "AllGather",
                    ALU.bypass,
                    replica_groups=rg,
                    ins=[agin[:]],
                    outs=[tfull[:]],
                )

            tables = (agin[:, :], tfull[:, :H], tfull[:, H:])
            steps = (None, 2 * H, 2 * H)
            vpools = (vlc_p, vlo_p, vhi_p)
            # chunked gathers over the dense slot streams; the stream-final
            # call skips trailing slots via num_idxs, so its last tile is
            # memset first (w=0 matmul columns must not hit NaN garbage)
            vchunks = [[None] * n_chunk[s] for s in range(3)]
            for _, s, ci in chunk_order:
                nt = min(C_TILES, NT[s] - ci * C_TILES)
                nidx = min(nt * 128, ST[s] - ci * C_TILES * 128)
                v = vpools[s].tile([128, C_TILES, 128], BF, tag=f"v{s}")
                if nidx < nt * 128:
                    nc.vector.memset(v[:, nt - 1, :], 0.0)
                nc.gpsimd.dma_gather(
                    v[:, :nt, :], tables[s],
                    idx_sb[s][:, ci * C_TILES * 8 : (ci * C_TILES + nt) * 8],
                    nidx, nidx, H, elem_step=steps[s], single_packet=False,
                )
                vchunks[s][ci] = v

            # per-block scatter-accumulate + epilogue; the self tile (SBUF
            # node-major t) leads each block's accumulation group.
            for b in range(NBLK):
                ntile = 1 + len(uses[b])
                agg = ps_blk.tile([128, BW], FP, tag="agg")
                sw = sw_tile(meta_of[b])
                nc.tensor.matmul(
                    agg[:], tt_sb[:, b * BW : (b + 1) * BW], sw[:],
                    start=True, stop=(ntile == 1),
                )
                for t, (s, T) in enumerate(uses[b], start=1):
                    sw = sw_tile(meta_of[b] + t)
                    v = vchunks[s][T // C_TILES][:, T % C_TILES, :]
                    nc.tensor.matmul(
                        agg[:], v, sw[:],
                        start=False, stop=(t == ntile - 1),
                    )
                rb = epp.tile([128, BW], BF, tag="rb")
                nmb = epp.tile([128, BW], FP, tag="nmb")
                eb = epp.tile([128, BW], BF, tag="eb")
                nc.scalar.activation(
                    rb[:], agg[:], ACT_F.Relu, bias=bias[:, bcol : bcol + 1]
                )
                nc.scalar.activation(
                    nmb[:], agg[:], ACT_F.Relu,
                    bias=bias[:, bcol + 7 : bcol + 8], scale=-1.0,
                )
                nc.scalar.activation(eb[:], nmb[:], ACT_F.Exp, scale=-1.0)
                nc.vector.scalar_tensor_tensor(
                    h_sb[:, b * BW : (b + 1) * BW],
                    eb[:], -1.0, rb[:], ALU.add, ALU.add,
                )

        # ---- head ----------------------------------------------------------
        for off, cw in CHUNKS:
            cw = min(cw, NC_N - off)
            ps = ps_dense.tile([128, 512], FP, tag="dense")
            nc.tensor.matmul(
                ps[:OUT_D, :cw], w_sb["Wh"][:], h_sb[:, off : off + cw]
            )
            nc.scalar.activation(
                oc[:, off : off + cw], ps[:OUT_D, :cw], ACT_F.Identity,
                bias=bias[:OUT_D, 14:15],
            )
        nc.sync.dma_start(out_d[:, :], oc[:, :NC_N])

    nc.compile()
    return nc


def _make_in_maps(inputs, per_core):
    import ml_dtypes

    x = np.asarray(inputs["x"], dtype=np.float32)
    bias = np.zeros((128, 16), dtype=np.float32)
    for j, nm in enumerate(["b1", "b2", "b3", "bg1", "bg2", "bg3", "bg4"]):
        b = np.asarray(inputs[nm], dtype=np.float32)
        bias[:, j] = b
        bias[:, j + 7] = -b
    bias[:OUT_D, 14] = np.asarray(inputs["bh"], dtype=np.float32)

    shared = {
        "bias": bias,
        "iota128": np.tile(
            np.arange(BW, dtype=np.float32), (128, 1)
        ).astype(ml_dtypes.bfloat16),
    }
    for nm in ["W1", "W2", "W3", "Wg1", "Wg2", "Wg3", "Wg4", "Wh"]:
        shared[nm] = np.ascontiguousarray(
            np.asarray(inputs[nm], np.float32)
        ).astype(ml_dtypes.bfloat16)

    in_maps = []
    for c in range(P):
        m = dict(shared)
        m["xT"] = np.ascontiguousarray(
            x[c * NC_N : (c + 1) * NC_N].T
        ).astype(ml_dtypes.bfloat16)
        m.update(per_core[c])
        in_maps.append(m)
    return in_maps


def run(inputs, trace=False):
    """Run the distributed kernel; returns (out [N, OUT_D] fp32, results)."""
    tcnt, per_core = _prep_edges(inputs["edge_index"], inputs["edge_weight"])
    nc = _build_program(tcnt)
    in_maps = _make_in_maps(inputs, per_core)
    res = run_bass_kernel_spmd(nc, in_maps, list(range(P)), trace=trace)
    out = np.concatenate(
        [res.results[c]["out"].T for c in range(P)], axis=0
    ).astype(np.float32)
    return out, res


def kernel(**inputs):
    out, _ = run(inputs, trace=False)
    return out


# revision 27
# speedup vs baseline: 1.4941x; 1.0280x over previous
"""GCN (4-layer, improved self-loops) on 8 Trainium2 NeuronCores.

Sharding: 1D node partition (6250 nodes/core); edges partitioned by
destination-node owner; per layer the raw features t = h@Wg are AllGathered
into a full bf16 DRAM table on every core, then each core gathers per-edge
source rows with dma_gather and scatter-adds them into per-destination-block
PSUM tiles via one-hot matmuls on the TensorEngine.

The full GCN normalization (w_e * dinv[src] * dinv[dst], and the self-loop
coefficient 2*dinv^2) is folded into the per-edge one-hot weights on the
host, so no on-chip pre/post scaling is needed:
    h_next = elu(sum_e w'_e * t[src_e] + b)
Self-loop contributions use the node-major t tiles already resident in SBUF
as scatter lhsT directly (no DMA gather, no table read).

The t table uses a permuted row layout (row = (m%128)*NB + m//128 for local
node m) so the whole per-layer table emit is one flat SBUF->DRAM copy of the
node-major tile; gather indices bake the permutation in on the host.  Gather
indices are int16, so the gathered table is split at row 32768 into lo/hi
streams.  Everything on the PE runs bf16 (1 cycle/row); destination blocks
are 128 wide to halve PE/DVE cost per edge vs 256-wide blocks.
"""

import numpy as np
from contextlib import ExitStack

try:
    import concourse.bass as bass
except ImportError:  # pragma: no cover
    import sys

    sys.path.insert(0, "/opt/trn_rl_repo")
    import concourse.bass as bass

import concourse.bacc as bacc
import concourse.mybir as mybir
import concourse.tile as tile
from concourse.bass_utils import run_bass_kernel_spmd

FP = mybir.dt.float32
BF = mybir.dt.bfloat16
I16 = mybir.dt.int16

N = 50000
E = 800000
IN_D = 64
H = 128
OUT_D = 16
P = 8
NC_N = N // P            # 6250 nodes per core
BW = 128                 # destination-block width (scatter matmul moving dim)
NBLK = -(-NC_N // BW)    # 49 destination blocks per core
NPAD = NBLK * BW         # 6272
SPLIT = 32768            # lo/hi split of permuted tfull rows (int16 indices)
C_TILES = 32             # 128-edge tiles per dma_gather call

# dense-matmul column chunks over the padded node dim
CHUNKS = [(k * 512, 512) for k in range(12)] + [(6144, 128)]

ALU = mybir.AluOpType
ACT_F = mybir.ActivationFunctionType


def _rowperm_local(m):
    """Permuted row index of local node m in the [NPAD, H] table (the flat
    view of the node-major [128, NBLK*H] SBUF tile)."""
    return (m % BW) * NBLK + m // BW


def _layout(nmax):
    """Dense per-stream slot layout shared by host prep and program build.

    Each (block, stream) group occupies slots [O[s][b], O[s][b]+nmax[s,b])
    of its stream — no tile alignment, so gather descriptors cover only the
    max-over-cores real edge count.  A 128-slot tile straddling a block
    boundary is consumed by both blocks (with w=0 masks from meta).

    Returns (O, ST, NT, uses, meta_of, TT): slot offsets, stream slot
    totals, stream tile counts, per-block list of (stream, tile) uses, the
    meta tile index of each block's self tile, and the total meta tiles.
    """
    O = [np.concatenate([[0], np.cumsum(nmax[s])]) for s in range(3)]
    ST = [int(O[s][-1]) for s in range(3)]
    NT = [-(-ST[s] // 128) for s in range(3)]
    uses = []
    meta_of = []
    om = 0
    for b in range(NBLK):
        meta_of.append(om)
        om += 1
        ub = []
        for s in range(3):
            n = int(nmax[s][b])
            if n == 0:
                continue
            t0 = int(O[s][b]) // 128
            t1 = int(O[s][b] + n - 1) // 128
            ub.extend((s, t) for t in range(t0, t1 + 1))
        om += len(ub)
        uses.append(ub)
    return O, ST, NT, uses, meta_of, om


def _prep_edges(edge_index, edge_weight):
    """Host preprocessing: partition edges by dst owner, fold the full GCN
    normalization into per-edge weights, split local/remote-lo/remote-hi by
    source table row, group by BW-dst block, pad each (core, block, stream)
    group to a common (max-over-cores) tile count.

    Returns (tcnt, per_core); per_core[c] has lcidx/loidx/hiidx/meta arrays.
    """
    import ml_dtypes

    src = np.asarray(edge_index[0], dtype=np.int64)
    dst = np.asarray(edge_index[1], dtype=np.int64)
    w = np.asarray(edge_weight, dtype=np.float32)

    core = dst // NC_N
    drel = dst % NC_N

    deg_full = np.zeros(N, dtype=np.float64)
    np.add.at(deg_full, dst, w.astype(np.float64))
    dinv_full = (1.0 / np.sqrt(deg_full + 2.0)).astype(np.float32)

    wn = w * dinv_full[src] * dinv_full[dst]   # folded edge norm

    # permuted global table row for source node s
    src_core = src // NC_N
    src_m = src % NC_N
    row_global = src_core * NPAD + (src_m % BW) * NBLK + src_m // BW
    row_local = (src_m % BW) * NBLK + src_m // BW

    groups = [[[None] * 3 for _ in range(NBLK)] for _ in range(P)]
    for c in range(P):
        mask = core == c
        s_core, d_all, w_all = src_core[mask], drel[mask], wn[mask]
        rg, rl = row_global[mask], row_local[mask]
        blk = d_all // BW
        rel = (d_all % BW).astype(np.float32)
        is_local = s_core == c
        even = rg % 2 == 0
        for b in range(NBLK):
            mb = blk == b
            # remote edges split by table-row parity: parity is uniform
            # across cores, so the max-over-cores stream sizes stay balanced
            # (an absolute row split would skew per core); idx = row//2 fits
            # int16 since the table is viewed as [rows/2, 2H] row pairs.
            for s, ms in (
                (0, mb & is_local),
                (1, mb & ~is_local & even),
                (2, mb & ~is_local & ~even),
            ):
                idx = rl[ms] if s == 0 else rg[ms] // 2
                # sort by source row for DRAM locality
                o = np.argsort(idx, kind="stable")
                groups[c][b][s] = (
                    idx[o].astype(np.int16), rel[ms][o], w_all[ms][o],
                )

    nmax = np.zeros((3, NBLK), dtype=np.int64)
    for b in range(NBLK):
        for s in range(3):
            for c in range(P):
                nmax[s, b] = max(nmax[s, b], len(groups[c][b][s][0]))

    O, ST, NT, uses, meta_of, TT = _layout(nmax)

    per_core = []
    for c in range(P):
        dinv_c = np.zeros(NPAD, dtype=np.float32)
        dinv_c[:NC_N] = dinv_full[c * NC_N : (c + 1) * NC_N]
        selfw = 2.0 * dinv_c * dinv_c

        idx_bufs = [np.zeros(NT[s] * 128, dtype=np.int16) for s in range(3)]
        # meta[e] = (dst_rel, w) per consumption-order tile USE: per block,
        # the self tile first, then each stream tile overlapping the block's
        # dense slot range; out-of-range rows keep w=0.
        meta_rel = np.zeros(TT * 128, dtype=np.float32)
        meta_w = np.zeros(TT * 128, dtype=np.float32)
        om = 0
        iota128 = np.arange(128, dtype=np.float32)
        for b in range(NBLK):
            meta_rel[om * 128 : om * 128 + 128] = iota128
            meta_w[om * 128 : om * 128 + 128] = selfw[b * 128 : (b + 1) * 128]
            om += 1
            for s, T in uses[b]:
                idx, rel, ww = groups[c][b][s]
                n = len(idx)
                o_sb = int(O[s][b])
                if T == o_sb // 128:
                    idx_bufs[s][o_sb : o_sb + n] = idx
                lo_s = max(o_sb, 128 * T)
                hi_s = min(o_sb + n, 128 * (T + 1))
                if hi_s > lo_s:
                    mo = om * 128 + (lo_s - 128 * T)
                    meta_rel[mo : mo + hi_s - lo_s] = rel[lo_s - o_sb : hi_s - o_sb]
                    meta_w[mo : mo + hi_s - lo_s] = ww[lo_s - o_sb : hi_s - o_sb]
                om += 1

        # wrapped int16 index layout: idx i lives at [i % 16, i // 16],
        # replicated 8x along partitions (one stripe per Q7 core)
        wraps = [
            np.ascontiguousarray(np.tile(ib.reshape(-1, 16).T, (8, 1)))
            if len(ib)
            else np.zeros((128, 0), dtype=np.int16)
            for ib in idx_bufs
        ]
        # meta in partition-major tile layout: edge t*128+p -> [p, 2t + {0,1}]
        meta = np.empty((128, 2 * TT), dtype=np.float32)
        meta[:, 0::2] = meta_rel.reshape(TT, 128).T
        meta[:, 1::2] = meta_w.reshape(TT, 128).T

        per_core.append(
            {
                "lcidx": wraps[0],
                "loidx": wraps[1],
                "hiidx": wraps[2],
                "meta": meta,
            }
        )

    return nmax, per_core


def _build_program(tcnt, single_core=False):
    # single_core=True swaps the AllGather for a local DMA copy and builds a
    # 1-device module, so the cost-model TimelineSim (single-core only) can
    # profile the kernel; numerics of remote nodes are wrong in that mode.
    nmax = tcnt
    O, ST, NT, uses, meta_of, TT = _layout(nmax)
    TS = NT
    nc = bacc.Bacc(
        "TRN2",
        target_bir_lowering=False,
        debug=False,
        enable_asserts=False,
        num_devices=1 if single_core else P,
    )

    # ---- I/O -------------------------------------------------------------
    xT_d = nc.dram_tensor("xT", [IN_D, NC_N], BF, kind="ExternalInput")
    lcidx_d = nc.dram_tensor("lcidx", [128, max(TS[0], 1) * 8], I16, kind="ExternalInput")
    loidx_d = nc.dram_tensor("loidx", [128, max(TS[1], 1) * 8], I16, kind="ExternalInput")
    hiidx_d = nc.dram_tensor("hiidx", [128, max(TS[2], 1) * 8], I16, kind="ExternalInput")
    meta_d = nc.dram_tensor("meta", [128, 2 * TT], FP, kind="ExternalInput")
    w_d = {
        name: nc.dram_tensor(name, shape, BF, kind="ExternalInput")
        for name, shape in [
            ("W1", [IN_D, H]),
            ("W2", [H, H]),
            ("W3", [H, H]),
            ("Wg1", [H, H]),
            ("Wg2", [H, H]),
            ("Wg3", [H, H]),
            ("Wg4", [H, H]),
            ("Wh", [H, OUT_D]),
        ]
    }
    # bias columns: 0..2 = b1..b3, 3..6 = bg1..bg4, 7..13 = negated, 14 = bh
    bias_d = nc.dram_tensor("bias", [128, 24], FP, kind="ExternalInput")
    iota_d = nc.dram_tensor("iota128", [128, BW], BF, kind="ExternalInput")
    out_d = nc.dram_tensor("out", [OUT_D, NC_N], FP, kind="ExternalOutput")

    rg = [list(range(P))]

    with tile.TileContext(nc) as tc, ExitStack() as ctx:
        const = ctx.enter_context(tc.tile_pool(name="const", bufs=1))
        big = ctx.enter_context(tc.tile_pool(name="big", bufs=1))
        swp = ctx.enter_context(tc.tile_pool(name="swp", bufs=48))
        epp = ctx.enter_context(tc.tile_pool(name="epp", bufs=3))
        rp_p = ctx.enter_context(tc.tile_pool(name="rp", bufs=13))
        e2_p = ctx.enter_context(tc.tile_pool(name="e2", bufs=13))
        vlc_p = ctx.enter_context(tc.tile_pool(name="vlc", bufs=3))
        vlo_p = ctx.enter_context(tc.tile_pool(name="vlo", bufs=3))
        vhi_p = ctx.enter_context(tc.tile_pool(name="vhi", bufs=3))
        ps_dense = ctx.enter_context(tc.tile_pool(name="psd", bufs=3, space="PSUM"))
        ps_blk = ctx.enter_context(tc.tile_pool(name="psb", bufs=3, space="PSUM"))
        ps_tr = ctx.enter_context(tc.tile_pool(name="pst", bufs=2, space="PSUM"))
        dram = ctx.enter_context(tc.tile_pool(name="dram", bufs=2, space="DRAM"))

        # ---- constants ----------------------------------------------------
        def load_const(shape, src_ap, name, dtype=FP):
            t = const.tile(shape, dtype, tag=name)
            nc.sync.dma_start(t[:], src_ap)
            return t

        w_sb = {k: load_const(list(v.shape), v[:], k, BF) for k, v in w_d.items()}
        bias = load_const([128, 24], bias_d[:], "bias")
        iota = load_const([128, BW], iota_d[:], "iota", BF)
        meta_sb = load_const([128, 2 * TT], meta_d[:], "meta")
        idx_sb = [
            load_const([128, max(TS[s], 1) * 8], d[:], f"idx{s}", I16)
            for s, d in enumerate((lcidx_d, loidx_d, hiidx_d))
        ]

        h_sb = big.tile([128, NPAD], BF, tag="h")
        tt_sb = big.tile([128, NPAD], BF, tag="tt")
        xc = big.tile([IN_D, NPAD], BF, tag="xc")
        oc = big.tile([OUT_D, NPAD], FP, tag="oc")

        nc.vector.memset(xc[:, NC_N:], 0.0)
        nc.sync.dma_start(xc[:, :NC_N], xT_d[:])

        def sw_tile(g):
            """[128 edge, BW dst] one-hot(dst_rel)*w scatter tile for
            consumption-order tile g, built on the vector engine."""
            sw = swp.tile([128, BW], BF, tag="sw")
            nc.vector.tensor_scalar(
                sw[:],
                iota[:],
                meta_sb[:, 2 * g : 2 * g + 1],
                meta_sb[:, 2 * g + 1 : 2 * g + 2],
                ALU.is_equal,
                ALU.mult,
            )
            return sw

        # ---- embedding MLP -------------------------------------------------

        # ELU via  elu(z) = min(exp(z), 1) + max(z-1, -1)  (exact for all z):
        # exp on ACT, the shifted relu on the idle Pool engine, combine on
        # DVE.  Stage-split loops keep each in-order engine queue free of
        # cross-chunk dependency chains.
        for wname, bcol in [("W1", 0), ("W2", 1), ("W3", 2)]:
            rps, e2s = [], []
            for off, cw in CHUNKS:
                ps = ps_dense.tile([128, 512], FP, tag="dense")
                if wname == "W1":
                    nc.tensor.matmul(
                        ps[:, :cw], w_sb["W1"][:IN_D, :], xc[:IN_D, off : off + cw]
                    )
                else:
                    nc.tensor.matmul(
                        ps[:, :cw], w_sb[wname][:], h_sb[:, off : off + cw]
                    )
                rp = rp_p.tile([128, 512], BF, tag="rp")
                nc.vector.tensor_scalar(
                    rp[:, :cw], ps[:, :cw], bias[:, bcol + 16 : bcol + 17],
                    -1.0, ALU.add, ALU.max,
                )
                e2 = e2_p.tile([128, 512], BF, tag="e2")
                nc.scalar.activation(
                    e2[:, :cw], ps[:, :cw], ACT_F.Exp,
                    bias=bias[:, bcol : bcol + 1],
                )
                rps.append(rp)
                e2s.append(e2)
            for ci, (off, cw) in enumerate(CHUNKS):
                nc.vector.scalar_tensor_tensor(
                    h_sb[:, off : off + cw], e2s[ci][:, :cw], 1.0,
                    rps[ci][:, :cw], ALU.min, ALU.add,
                )

        # ---- GCN layers ---------------------------------------------------
        n_chunk = [-(-NT[s] // C_TILES) for s in range(3)]
        # emit gather chunks interleaved by first consuming block
        chunk_order = sorted(
            (max(0, int(np.searchsorted(O[s], ci * C_TILES * 128, "right")) - 1),
             s, ci)
            for s in range(3) for ci in range(n_chunk[s])
        )

        for layer in range(4):
            wg = w_sb[f"Wg{layer + 1}"]
            bcol = 3 + layer

            # t (node-major) = h_blk^T @ Wg per block; lhsT = h slice puts
            # nodes on the output partition axis, so no transposes needed.
            for b in range(NBLK):
                trp = ps_tr.tile([128, 128], FP, tag="tr")
                nc.tensor.matmul(trp[:], h_sb[:, b * BW : (b + 1) * BW], wg[:])
                nc.scalar.activation(
                    tt_sb[:, b * BW : (b + 1) * BW], trp[:], ACT_F.Copy
                )

            # one flat table emit; rows are host-side permuted.  tfull is
            # viewed as [row-pairs, 2H] so the even/odd gather streams can
            # address all P*NPAD rows with int16 pair indices + elem_step.
            agin = dram.tile([NPAD, H], BF, tag="agin")
            tfull = dram.tile(
                [P * NPAD // 2, 2 * H], BF, tag="tfull", addr_space="Shared"
            )
            nc.sync.dma_start(agin[:, :], tt_sb[:])

            if single_core:
                nc.sync.dma_start(tfull[: NPAD // 2, :], agin[:, :])
            else:
                nc.gpsimd.collective_compute(
                    "AllGather",
                    ALU.bypass,
                    replica_groups=rg,
                    ins=[agin[:]],
                    outs=[tfull[:]],
                )

            tables = (agin[:, :], tfull[:, :H], tfull[:, H:])
            steps = (None, 2 * H, 2 * H)
            vpools = (vlc_p, vlo_p, vhi_p)
            # chunked gathers over the dense slot streams; the stream-final
            # call skips trailing slots via num_idxs, so its last tile is
            # memset first (w=0 matmul columns must not hit NaN garbage)
            vchunks = [[None] * n_chunk[s] for s in range(3)]
            for _, s, ci in chunk_order:
                nt = min(C_TILES, NT[s] - ci * C_TILES)
                nidx = min(nt * 128, ST[s] - ci * C_TILES * 128)
                v = vpools[s].tile([128, C_TILES, 128], BF, tag=f"v{s}")
                if nidx < nt * 128:
                    nc.vector.memset(v[:, nt - 1, :], 0.0)
                nc.gpsimd.dma_gather(
                    v[:, :nt, :], tables[s],
                    idx_sb[s][:, ci * C_TILES * 8 : (ci * C_TILES + nt) * 8],
                    nidx, nidx, H, elem_step=steps[s], single_packet=False,
                )
                vchunks[s][ci] = v

            # per-block scatter-accumulate + epilogue; the self tile (SBUF
            # node-major t) leads each block's accumulation group.
            for b in range(NBLK):
                ntile = 1 + len(uses[b])
                agg = ps_blk.tile([128, BW], FP, tag="agg")
                sw = sw_tile(meta_of[b])
                nc.tensor.matmul(
                    agg[:], tt_sb[:, b * BW : (b + 1) * BW], sw[:],
                    start=True, stop=(ntile == 1),
                )
                for t, (s, T) in enumerate(uses[b], start=1):
                    sw = sw_tile(meta_of[b] + t)
                    v = vchunks[s][T // C_TILES][:, T % C_TILES, :]
                    nc.tensor.matmul(
                        agg[:], v, sw[:],
                        start=False, stop=(t == ntile - 1),
                    )
                rp = epp.tile([128, BW], FP, tag="rpb")
                nc.vector.tensor_scalar(
                    rp[:], agg[:], bias[:, bcol + 16 : bcol + 17],
                    -1.0, ALU.add, ALU.max,
                )
                eb = epp.tile([128, BW], BF, tag="eb")
                nc.scalar.activation(
                    eb[:], agg[:], ACT_F.Exp, bias=bias[:, bcol : bcol + 1]
                )
                nc.vector.scalar_tensor_tensor(
                    h_sb[:, b * BW : (b + 1) * BW],
                    eb[:], 1.0, rp[:], ALU.min, ALU.add,
                )

        # ---- head ----------------------------------------------------------
        for off, cw in CHUNKS:
            cw = min(cw, NC_N - off)
            ps = ps_dense.tile([128, 512], FP, tag="dense")
            nc.tensor.matmul(
                ps[:OUT_D, :cw], w_sb["Wh"][:], h_sb[:, off : off + cw]
            )
            nc.scalar.activation(
                oc[:, off : off + cw], ps[:OUT_D, :cw], ACT_F.Identity,
                bias=bias[:OUT_D, 14:15],
            )
        nc.sync.dma_start(out_d[:, :], oc[:, :NC_N])

    nc.compile()
    return nc


def _make_in_maps(inputs, per_core):
    import ml_dtypes

    x = np.asarray(inputs["x"], dtype=np.float32)
    bias = np.zeros((128, 24), dtype=np.float32)
    for j, nm in enumerate(["b1", "b2", "b3", "bg1", "bg2", "bg3", "bg4"]):
        b = np.asarray(inputs[nm], dtype=np.float32)
        bias[:, j] = b
        bias[:, j + 16] = b - 1.0
    bias[:OUT_D, 14] = np.asarray(inputs["bh"], dtype=np.float32)

    shared = {
        "bias": bias,
        "iota128": np.tile(
            np.arange(BW, dtype=np.float32), (128, 1)
        ).astype(ml_dtypes.bfloat16),
    }
    for nm in ["W1", "W2", "W3", "Wg1", "Wg2", "Wg3", "Wg4", "Wh"]:
        shared[nm] = np.ascontiguousarray(
            np.asarray(inputs[nm], np.float32)
        ).astype(ml_dtypes.bfloat16)

    in_maps = []
    for c in range(P):
        m = dict(shared)
        m["xT"] = np.ascontiguousarray(
            x[c * NC_N : (c + 1) * NC_N].T
        ).astype(ml_dtypes.bfloat16)
        m.update(per_core[c])
        in_maps.append(m)
    return in_maps


def run(inputs, trace=False):
    """Run the distributed kernel; returns (out [N, OUT_D] fp32, results)."""
    tcnt, per_core = _prep_edges(inputs["edge_index"], inputs["edge_weight"])
    nc = _build_program(tcnt)
    in_maps = _make_in_maps(inputs, per_core)
    res = run_bass_kernel_spmd(nc, in_maps, list(range(P)), trace=trace)
    out = np.concatenate(
        [res.results[c]["out"].T for c in range(P)], axis=0
    ).astype(np.float32)
    return out, res


def kernel(**inputs):
    out, _ = run(inputs, trace=False)
    return out


# revision 28
# speedup vs baseline: 1.5075x; 1.0090x over previous
"""GCN (4-layer, improved self-loops) on 8 Trainium2 NeuronCores.

Sharding: 1D node partition (6250 nodes/core); edges partitioned by
destination-node owner; per layer the raw features t = h@Wg are AllGathered
into a full bf16 DRAM table on every core, then each core gathers per-edge
source rows with dma_gather and scatter-adds them into per-destination-block
PSUM tiles via one-hot matmuls on the TensorEngine.

The full GCN normalization (w_e * dinv[src] * dinv[dst], and the self-loop
coefficient 2*dinv^2) is folded into the per-edge one-hot weights on the
host, so no on-chip pre/post scaling is needed:
    h_next = elu(sum_e w'_e * t[src_e] + b)
Self-loop contributions use the node-major t tiles already resident in SBUF
as scatter lhsT directly (no DMA gather, no table read).

The t table uses a permuted row layout (row = (m%128)*NB + m//128 for local
node m) so the whole per-layer table emit is one flat SBUF->DRAM copy of the
node-major tile; gather indices bake the permutation in on the host.  Gather
indices are int16, so the gathered table is split at row 32768 into lo/hi
streams.  Everything on the PE runs bf16 (1 cycle/row); destination blocks
are 128 wide to halve PE/DVE cost per edge vs 256-wide blocks.
"""

import numpy as np
from contextlib import ExitStack

try:
    import concourse.bass as bass
except ImportError:  # pragma: no cover
    import sys

    sys.path.insert(0, "/opt/trn_rl_repo")
    import concourse.bass as bass

import concourse.bacc as bacc
import concourse.mybir as mybir
import concourse.tile as tile
from concourse.bass_utils import run_bass_kernel_spmd

FP = mybir.dt.float32
BF = mybir.dt.bfloat16
I16 = mybir.dt.int16

N = 50000
E = 800000
IN_D = 64
H = 128
OUT_D = 16
P = 8
NC_N = N // P            # 6250 nodes per core
BW = 128                 # destination-block width (scatter matmul moving dim)
NBLK = -(-NC_N // BW)    # 49 destination blocks per core
NPAD = NBLK * BW         # 6272
SPLIT = 32768            # lo/hi split of permuted tfull rows (int16 indices)
C_TILES = 32             # 128-edge tiles per dma_gather call

# dense-matmul column chunks over the padded node dim
CHUNKS = [(k * 512, 512) for k in range(12)] + [(6144, 128)]

ALU = mybir.AluOpType
ACT_F = mybir.ActivationFunctionType


def _rowperm_local(m):
    """Permuted row index of local node m in the [NPAD, H] table (the flat
    view of the node-major [128, NBLK*H] SBUF tile)."""
    return (m % BW) * NBLK + m // BW


def _layout(nmax):
    """Dense per-stream slot layout shared by host prep and program build.

    Each (block, stream) group occupies slots [O[s][b], O[s][b]+nmax[s,b])
    of its stream — no tile alignment, so gather descriptors cover only the
    max-over-cores real edge count.  A 128-slot tile straddling a block
    boundary is consumed by both blocks (with w=0 masks from meta).

    Returns (O, ST, NT, uses, meta_of, TT): slot offsets, stream slot
    totals, stream tile counts, per-block list of (stream, tile) uses, the
    meta tile index of each block's self tile, and the total meta tiles.
    """
    O = [np.concatenate([[0], np.cumsum(nmax[s])]) for s in range(3)]
    ST = [int(O[s][-1]) for s in range(3)]
    NT = [-(-ST[s] // 128) for s in range(3)]
    uses = []
    meta_of = []
    om = 0
    for b in range(NBLK):
        meta_of.append(om)
        om += 1
        ub = []
        for s in range(3):
            n = int(nmax[s][b])
            if n == 0:
                continue
            t0 = int(O[s][b]) // 128
            t1 = int(O[s][b] + n - 1) // 128
            ub.extend((s, t) for t in range(t0, t1 + 1))
        om += len(ub)
        uses.append(ub)
    return O, ST, NT, uses, meta_of, om


def _prep_edges(edge_index, edge_weight):
    """Host preprocessing: partition edges by dst owner, fold the full GCN
    normalization into per-edge weights, split local/remote-lo/remote-hi by
    source table row, group by BW-dst block, pad each (core, block, stream)
    group to a common (max-over-cores) tile count.

    Returns (tcnt, per_core); per_core[c] has lcidx/loidx/hiidx/meta arrays.
    """
    import ml_dtypes

    src = np.asarray(edge_index[0], dtype=np.int64)
    dst = np.asarray(edge_index[1], dtype=np.int64)
    w = np.asarray(edge_weight, dtype=np.float32)

    core = dst // NC_N
    drel = dst % NC_N

    deg_full = np.zeros(N, dtype=np.float64)
    np.add.at(deg_full, dst, w.astype(np.float64))
    dinv_full = (1.0 / np.sqrt(deg_full + 2.0)).astype(np.float32)

    wn = w * dinv_full[src] * dinv_full[dst]   # folded edge norm

    # permuted global table row for source node s
    src_core = src // NC_N
    src_m = src % NC_N
    row_global = src_core * NPAD + (src_m % BW) * NBLK + src_m // BW
    row_local = (src_m % BW) * NBLK + src_m // BW

    groups = [[[None] * 3 for _ in range(NBLK)] for _ in range(P)]
    for c in range(P):
        mask = core == c
        s_core, d_all, w_all = src_core[mask], drel[mask], wn[mask]
        rg, rl = row_global[mask], row_local[mask]
        blk = d_all // BW
        rel = (d_all % BW).astype(np.float32)
        is_local = s_core == c
        even = rg % 2 == 0
        for b in range(NBLK):
            mb = blk == b
            # remote edges split by table-row parity: parity is uniform
            # across cores, so the max-over-cores stream sizes stay balanced
            # (an absolute row split would skew per core); idx = row//2 fits
            # int16 since the table is viewed as [rows/2, 2H] row pairs.
            for s, ms in (
                (0, mb & is_local),
                (1, mb & ~is_local & even),
                (2, mb & ~is_local & ~even),
            ):
                idx = rl[ms] if s == 0 else rg[ms] // 2
                # sort by source row for DRAM locality
                o = np.argsort(idx, kind="stable")
                groups[c][b][s] = (
                    idx[o].astype(np.int16), rel[ms][o], w_all[ms][o],
                )

    nmax = np.zeros((3, NBLK), dtype=np.int64)
    for b in range(NBLK):
        for s in range(3):
            for c in range(P):
                nmax[s, b] = max(nmax[s, b], len(groups[c][b][s][0]))

    O, ST, NT, uses, meta_of, TT = _layout(nmax)

    per_core = []
    for c in range(P):
        dinv_c = np.zeros(NPAD, dtype=np.float32)
        dinv_c[:NC_N] = dinv_full[c * NC_N : (c + 1) * NC_N]
        selfw = 2.0 * dinv_c * dinv_c

        idx_bufs = [np.zeros(NT[s] * 128, dtype=np.int16) for s in range(3)]
        # meta[e] = (dst_rel, w) per consumption-order tile USE: per block,
        # the self tile first, then each stream tile overlapping the block's
        # dense slot range; out-of-range rows keep w=0.
        meta_rel = np.zeros(TT * 128, dtype=np.float32)
        meta_w = np.zeros(TT * 128, dtype=np.float32)
        om = 0
        iota128 = np.arange(128, dtype=np.float32)
        for b in range(NBLK):
            meta_rel[om * 128 : om * 128 + 128] = iota128
            meta_w[om * 128 : om * 128 + 128] = selfw[b * 128 : (b + 1) * 128]
            om += 1
            for s, T in uses[b]:
                idx, rel, ww = groups[c][b][s]
                n = len(idx)
                o_sb = int(O[s][b])
                if T == o_sb // 128:
                    idx_bufs[s][o_sb : o_sb + n] = idx
                lo_s = max(o_sb, 128 * T)
                hi_s = min(o_sb + n, 128 * (T + 1))
                if hi_s > lo_s:
                    mo = om * 128 + (lo_s - 128 * T)
                    meta_rel[mo : mo + hi_s - lo_s] = rel[lo_s - o_sb : hi_s - o_sb]
                    meta_w[mo : mo + hi_s - lo_s] = ww[lo_s - o_sb : hi_s - o_sb]
                om += 1

        # wrapped int16 index layout: idx i lives at [i % 16, i // 16],
        # replicated 8x along partitions (one stripe per Q7 core)
        wraps = [
            np.ascontiguousarray(np.tile(ib.reshape(-1, 16).T, (8, 1)))
            if len(ib)
            else np.zeros((128, 0), dtype=np.int16)
            for ib in idx_bufs
        ]
        # meta in partition-major tile layout: edge t*128+p -> [p, 2t + {0,1}]
        meta = np.empty((128, 2 * TT), dtype=np.float32)
        meta[:, 0::2] = meta_rel.reshape(TT, 128).T
        meta[:, 1::2] = meta_w.reshape(TT, 128).T

        per_core.append(
            {
                "lcidx": wraps[0],
                "loidx": wraps[1],
                "hiidx": wraps[2],
                "meta": meta,
            }
        )

    return nmax, per_core


def _build_program(tcnt, single_core=False):
    # single_core=True swaps the AllGather for a local DMA copy and builds a
    # 1-device module, so the cost-model TimelineSim (single-core only) can
    # profile the kernel; numerics of remote nodes are wrong in that mode.
    nmax = tcnt
    O, ST, NT, uses, meta_of, TT = _layout(nmax)
    TS = NT
    nc = bacc.Bacc(
        "TRN2",
        target_bir_lowering=False,
        debug=False,
        enable_asserts=False,
        num_devices=1 if single_core else P,
    )

    # ---- I/O -------------------------------------------------------------
    xT_d = nc.dram_tensor("xT", [IN_D, NC_N], BF, kind="ExternalInput")
    lcidx_d = nc.dram_tensor("lcidx", [128, max(TS[0], 1) * 8], I16, kind="ExternalInput")
    loidx_d = nc.dram_tensor("loidx", [128, max(TS[1], 1) * 8], I16, kind="ExternalInput")
    hiidx_d = nc.dram_tensor("hiidx", [128, max(TS[2], 1) * 8], I16, kind="ExternalInput")
    meta_d = nc.dram_tensor("meta", [128, 2 * TT], FP, kind="ExternalInput")
    w_d = {
        name: nc.dram_tensor(name, shape, BF, kind="ExternalInput")
        for name, shape in [
            ("W1", [IN_D, H]),
            ("W2", [H, H]),
            ("W3", [H, H]),
            ("Wg1", [H, H]),
            ("Wg2", [H, H]),
            ("Wg3", [H, H]),
            ("Wg4", [H, H]),
            ("Wh", [H, OUT_D]),
        ]
    }
    # bias columns: 0..2 = b1..b3, 3..6 = bg1..bg4, 7..13 = negated, 14 = bh
    bias_d = nc.dram_tensor("bias", [128, 24], FP, kind="ExternalInput")
    iota_d = nc.dram_tensor("iota128", [128, BW], BF, kind="ExternalInput")
    out_d = nc.dram_tensor("out", [OUT_D, NC_N], FP, kind="ExternalOutput")

    rg = [list(range(P))]

    with tile.TileContext(nc) as tc, ExitStack() as ctx:
        const = ctx.enter_context(tc.tile_pool(name="const", bufs=1))
        big = ctx.enter_context(tc.tile_pool(name="big", bufs=1))
        swp = ctx.enter_context(tc.tile_pool(name="swp", bufs=80))
        epp = ctx.enter_context(tc.tile_pool(name="epp", bufs=3))
        rp_p = ctx.enter_context(tc.tile_pool(name="rp", bufs=13))
        e2_p = ctx.enter_context(tc.tile_pool(name="e2", bufs=13))
        vlc_p = ctx.enter_context(tc.tile_pool(name="vlc", bufs=3))
        vlo_p = ctx.enter_context(tc.tile_pool(name="vlo", bufs=3))
        vhi_p = ctx.enter_context(tc.tile_pool(name="vhi", bufs=3))
        ps_dense = ctx.enter_context(tc.tile_pool(name="psd", bufs=3, space="PSUM"))
        ps_blk = ctx.enter_context(tc.tile_pool(name="psb", bufs=3, space="PSUM"))
        ps_tr = ctx.enter_context(tc.tile_pool(name="pst", bufs=2, space="PSUM"))
        dram = ctx.enter_context(tc.tile_pool(name="dram", bufs=2, space="DRAM"))

        # ---- constants ----------------------------------------------------
        def load_const(shape, src_ap, name, dtype=FP):
            t = const.tile(shape, dtype, tag=name)
            nc.sync.dma_start(t[:], src_ap)
            return t

        w_sb = {k: load_const(list(v.shape), v[:], k, BF) for k, v in w_d.items()}
        bias = load_const([128, 24], bias_d[:], "bias")
        iota = load_const([128, BW], iota_d[:], "iota", BF)
        meta_sb = load_const([128, 2 * TT], meta_d[:], "meta")
        idx_sb = [
            load_const([128, max(TS[s], 1) * 8], d[:], f"idx{s}", I16)
            for s, d in enumerate((lcidx_d, loidx_d, hiidx_d))
        ]

        h_sb = big.tile([128, NPAD], BF, tag="h")
        tt_sb = big.tile([128, NPAD], BF, tag="tt")
        xc = big.tile([IN_D, NPAD], BF, tag="xc")
        oc = big.tile([OUT_D, NPAD], FP, tag="oc")

        nc.vector.memset(xc[:, NC_N:], 0.0)
        nc.sync.dma_start(xc[:, :NC_N], xT_d[:])

        def sw_tile(g):
            """[128 edge, BW dst] one-hot(dst_rel)*w scatter tile for
            consumption-order tile g, built on the vector engine."""
            sw = swp.tile([128, BW], BF, tag="sw")
            nc.vector.tensor_scalar(
                sw[:],
                iota[:],
                meta_sb[:, 2 * g : 2 * g + 1],
                meta_sb[:, 2 * g + 1 : 2 * g + 2],
                ALU.is_equal,
                ALU.mult,
            )
            return sw

        def emit_t(b0, b1, wg):
            # t (node-major) = h_blk^T @ Wg per block; lhsT = h slice puts
            # nodes on the output partition axis, so no transposes needed.
            for b in range(b0, b1):
                trp = ps_tr.tile([128, 128], FP, tag="tr")
                nc.tensor.matmul(trp[:], h_sb[:, b * BW : (b + 1) * BW], wg[:])
                nc.scalar.activation(
                    tt_sb[:, b * BW : (b + 1) * BW], trp[:], ACT_F.Copy
                )

        # ---- embedding MLP -------------------------------------------------

        # ELU via  elu(z) = min(exp(z), 1) + max(z-1, -1)  (exact for all z):
        # exp on ACT, the shifted relu and combine on DVE.  Stage-split loops
        # keep each in-order engine queue free of cross-chunk dependency
        # chains.  The W3 combine loop interleaves layer 1's t-matmuls so the
        # first GCN table emit isn't serialized behind the whole MLP.
        for wname, bcol in [("W1", 0), ("W2", 1), ("W3", 2)]:
            rps, e2s = [], []
            for off, cw in CHUNKS:
                ps = ps_dense.tile([128, 512], FP, tag="dense")
                if wname == "W1":
                    nc.tensor.matmul(
                        ps[:, :cw], w_sb["W1"][:IN_D, :], xc[:IN_D, off : off + cw]
                    )
                else:
                    nc.tensor.matmul(
                        ps[:, :cw], w_sb[wname][:], h_sb[:, off : off + cw]
                    )
                rp = rp_p.tile([128, 512], BF, tag="rp")
                nc.vector.tensor_scalar(
                    rp[:, :cw], ps[:, :cw], bias[:, bcol + 16 : bcol + 17],
                    -1.0, ALU.add, ALU.max,
                )
                e2 = e2_p.tile([128, 512], BF, tag="e2")
                nc.scalar.activation(
                    e2[:, :cw], ps[:, :cw], ACT_F.Exp,
                    bias=bias[:, bcol : bcol + 1],
                )
                rps.append(rp)
                e2s.append(e2)
            for ci, (off, cw) in enumerate(CHUNKS):
                nc.vector.scalar_tensor_tensor(
                    h_sb[:, off : off + cw], e2s[ci][:, :cw], 1.0,
                    rps[ci][:, :cw], ALU.min, ALU.add,
                )
                if wname == "W3":
                    emit_t(off // BW, min(NBLK, (off + cw) // BW), w_sb["Wg1"])

        # ---- GCN layers ---------------------------------------------------
        cstarts = []
        for s in range(3):
            sizes, rem = [], NT[s]
            while rem > C_TILES + C_TILES // 2:
                sizes.append(C_TILES)
                rem -= C_TILES
            if rem > C_TILES // 2:
                sizes.extend([(rem + 1) // 2, rem // 2])
            elif rem:
                sizes.append(rem)
            cstarts.append(np.concatenate([[0], np.cumsum(sizes)]).astype(int))
        n_chunk = [len(cstarts[s]) - 1 for s in range(3)]
        # emit gather chunks interleaved by first consuming block
        chunk_order = sorted(
            (max(0, int(np.searchsorted(O[s], int(cstarts[s][ci]) * 128,
                                        "right")) - 1), s, ci)
            for s in range(3) for ci in range(n_chunk[s])
        )

        for layer in range(4):
            wg = w_sb[f"Wg{layer + 1}"]
            bcol = 3 + layer

            if layer > 0:
                emit_t(0, NBLK, wg)

            # one flat table emit; rows are host-side permuted.  tfull is
            # viewed as [row-pairs, 2H] so the even/odd gather streams can
            # address all P*NPAD rows with int16 pair indices + elem_step.
            agin = dram.tile([NPAD, H], BF, tag="agin")
            tfull = dram.tile(
                [P * NPAD // 2, 2 * H], BF, tag="tfull", addr_space="Shared"
            )
            nc.sync.dma_start(agin[:, :], tt_sb[:])

            if single_core:
                nc.sync.dma_start(tfull[: NPAD // 2, :], agin[:, :])
            else:
                nc.gpsimd.collective_compute(
                    "AllGather",
                    ALU.bypass,
                    replica_groups=rg,
                    ins=[agin[:]],
                    outs=[tfull[:]],
                )

            tables = (agin[:, :], tfull[:, :H], tfull[:, H:])
            steps = (None, 2 * H, 2 * H)
            vpools = (vlc_p, vlo_p, vhi_p)
            # chunked gathers over the dense slot streams; the stream-final
            # call skips trailing slots via num_idxs, so its last tile is
            # memset first (w=0 matmul columns must not hit NaN garbage)
            vchunks = [[None] * n_chunk[s] for s in range(3)]
            for _, s, ci in chunk_order:
                t0 = int(cstarts[s][ci])
                nt = int(cstarts[s][ci + 1]) - t0
                nidx = min(nt * 128, ST[s] - t0 * 128)
                v = vpools[s].tile([128, C_TILES, 128], BF, tag=f"v{s}")
                if nidx < nt * 128:
                    nc.vector.memset(v[:, nt - 1, :], 0.0)
                nc.gpsimd.dma_gather(
                    v[:, :nt, :], tables[s],
                    idx_sb[s][:, t0 * 8 : (t0 + nt) * 8],
                    nidx, nidx, H, elem_step=steps[s], single_packet=False,
                )
                vchunks[s][ci] = v

            # per-block scatter-accumulate + epilogue; the self tile (SBUF
            # node-major t) leads each block's accumulation group.
            for b in range(NBLK):
                ntile = 1 + len(uses[b])
                agg = ps_blk.tile([128, BW], FP, tag="agg")
                sw = sw_tile(meta_of[b])
                nc.tensor.matmul(
                    agg[:], tt_sb[:, b * BW : (b + 1) * BW], sw[:],
                    start=True, stop=(ntile == 1),
                )
                for t, (s, T) in enumerate(uses[b], start=1):
                    sw = sw_tile(meta_of[b] + t)
                    ci = int(np.searchsorted(cstarts[s], T, "right")) - 1
                    v = vchunks[s][ci][:, T - int(cstarts[s][ci]), :]
                    nc.tensor.matmul(
                        agg[:], v, sw[:],
                        start=False, stop=(t == ntile - 1),
                    )
                rp = epp.tile([128, BW], FP, tag="rpb")
                nc.vector.tensor_scalar(
                    rp[:], agg[:], bias[:, bcol + 16 : bcol + 17],
                    -1.0, ALU.add, ALU.max,
                )
                eb = epp.tile([128, BW], BF, tag="eb")
                nc.scalar.activation(
                    eb[:], agg[:], ACT_F.Exp, bias=bias[:, bcol : bcol + 1]
                )
                nc.vector.scalar_tensor_tensor(
                    h_sb[:, b * BW : (b + 1) * BW],
                    eb[:], 1.0, rp[:], ALU.min, ALU.add,
                )

        # ---- head ----------------------------------------------------------
        for off, cw in CHUNKS:
            cw = min(cw, NC_N - off)
            ps = ps_dense.tile([128, 512], FP, tag="dense")
            nc.tensor.matmul(
                ps[:OUT_D, :cw], w_sb["Wh"][:], h_sb[:, off : off + cw]
            )
            nc.scalar.activation(
                oc[:, off : off + cw], ps[:OUT_D, :cw], ACT_F.Identity,
                bias=bias[:OUT_D, 14:15],
            )
        nc.sync.dma_start(out_d[:, :], oc[:, :NC_N])

    nc.compile()
    return nc


def _make_in_maps(inputs, per_core):
    import ml_dtypes

    x = np.asarray(inputs["x"], dtype=np.float32)
    bias = np.zeros((128, 24), dtype=np.float32)
    for j, nm in enumerate(["b1", "b2", "b3", "bg1", "bg2", "bg3", "bg4"]):
        b = np.asarray(inputs[nm], dtype=np.float32)
        bias[:, j] = b
        bias[:, j + 16] = b - 1.0
    bias[:OUT_D, 14] = np.asarray(inputs["bh"], dtype=np.float32)

    shared = {
        "bias": bias,
        "iota128": np.tile(
            np.arange(BW, dtype=np.float32), (128, 1)
        ).astype(ml_dtypes.bfloat16),
    }
    for nm in ["W1", "W2", "W3", "Wg1", "Wg2", "Wg3", "Wg4", "Wh"]:
        shared[nm] = np.ascontiguousarray(
            np.asarray(inputs[nm], np.float32)
        ).astype(ml_dtypes.bfloat16)

    in_maps = []
    for c in range(P):
        m = dict(shared)
        m["xT"] = np.ascontiguousarray(
            x[c * NC_N : (c + 1) * NC_N].T
        ).astype(ml_dtypes.bfloat16)
        m.update(per_core[c])
        in_maps.append(m)
    return in_maps


def run(inputs, trace=False):
    """Run the distributed kernel; returns (out [N, OUT_D] fp32, results)."""
    tcnt, per_core = _prep_edges(inputs["edge_index"], inputs["edge_weight"])
    nc = _build_program(tcnt)
    in_maps = _make_in_maps(inputs, per_core)
    res = run_bass_kernel_spmd(nc, in_maps, list(range(P)), trace=trace)
    out = np.concatenate(
        [res.results[c]["out"].T for c in range(P)], axis=0
    ).astype(np.float32)
    return out, res


def kernel(**inputs):
    out, _ = run(inputs, trace=False)
    return out


# revision 29
# speedup vs baseline: 1.5087x; 1.0008x over previous
"""GCN (4-layer, improved self-loops) on 8 Trainium2 NeuronCores.

Sharding: 1D node partition (6250 nodes/core); edges partitioned by
destination-node owner; per layer the raw features t = h@Wg are AllGathered
into a full bf16 DRAM table on every core, then each core gathers per-edge
source rows with dma_gather and scatter-adds them into per-destination-block
PSUM tiles via one-hot matmuls on the TensorEngine.

The full GCN normalization (w_e * dinv[src] * dinv[dst], and the self-loop
coefficient 2*dinv^2) is folded into the per-edge one-hot weights on the
host, so no on-chip pre/post scaling is needed:
    h_next = elu(sum_e w'_e * t[src_e] + b)
Self-loop contributions use the node-major t tiles already resident in SBUF
as scatter lhsT directly (no DMA gather, no table read).

The t table uses a permuted row layout (row = (m%128)*NB + m//128 for local
node m) so the whole per-layer table emit is one flat SBUF->DRAM copy of the
node-major tile; gather indices bake the permutation in on the host.  Gather
indices are int16, so the gathered table is split at row 32768 into lo/hi
streams.  Everything on the PE runs bf16 (1 cycle/row); destination blocks
are 128 wide to halve PE/DVE cost per edge vs 256-wide blocks.
"""

import numpy as np
from contextlib import ExitStack

try:
    import concourse.bass as bass
except ImportError:  # pragma: no cover
    import sys

    sys.path.insert(0, "/opt/trn_rl_repo")
    import concourse.bass as bass

import concourse.bacc as bacc
import concourse.mybir as mybir
import concourse.tile as tile
from concourse.bass_utils import run_bass_kernel_spmd

FP = mybir.dt.float32
BF = mybir.dt.bfloat16
I16 = mybir.dt.int16

N = 50000
E = 800000
IN_D = 64
H = 128
OUT_D = 16
P = 8
NC_N = N // P            # 6250 nodes per core
BW = 128                 # destination-block width (scatter matmul moving dim)
NBLK = -(-NC_N // BW)    # 49 destination blocks per core
NPAD = NBLK * BW         # 6272
SPLIT = 32768            # lo/hi split of permuted tfull rows (int16 indices)
C_TILES = 32             # 128-edge tiles per dma_gather call

# dense-matmul column chunks over the padded node dim
CHUNKS = [(k * 512, 512) for k in range(12)] + [(6144, 128)]

ALU = mybir.AluOpType
ACT_F = mybir.ActivationFunctionType


def _rowperm_local(m):
    """Permuted row index of local node m in the [NPAD, H] table (the flat
    view of the node-major [128, NBLK*H] SBUF tile)."""
    return (m % BW) * NBLK + m // BW


def _layout(nmax):
    """Dense per-stream slot layout shared by host prep and program build.

    Each (block, stream) group occupies slots [O[s][b], O[s][b]+nmax[s,b])
    of its stream — no tile alignment, so gather descriptors cover only the
    max-over-cores real edge count.  A 128-slot tile straddling a block
    boundary is consumed by both blocks (with w=0 masks from meta).

    Returns (O, ST, NT, uses, meta_of, TT): slot offsets, stream slot
    totals, stream tile counts, per-block list of (stream, tile) uses, the
    meta tile index of each block's self tile, and the total meta tiles.
    """
    O = [np.concatenate([[0], np.cumsum(nmax[s])]) for s in range(3)]
    ST = [int(O[s][-1]) for s in range(3)]
    NT = [-(-ST[s] // 128) for s in range(3)]
    uses = []
    meta_of = []
    om = 0
    for b in range(NBLK):
        meta_of.append(om)
        om += 1
        ub = []
        for s in range(3):
            n = int(nmax[s][b])
            if n == 0:
                continue
            t0 = int(O[s][b]) // 128
            t1 = int(O[s][b] + n - 1) // 128
            ub.extend((s, t) for t in range(t0, t1 + 1))
        om += len(ub)
        uses.append(ub)
    return O, ST, NT, uses, meta_of, om


def _prep_edges(edge_index, edge_weight):
    """Host preprocessing: partition edges by dst owner, fold the full GCN
    normalization into per-edge weights, split local/remote-lo/remote-hi by
    source table row, group by BW-dst block, pad each (core, block, stream)
    group to a common (max-over-cores) tile count.

    Returns (tcnt, per_core); per_core[c] has lcidx/loidx/hiidx/meta arrays.
    """
    import ml_dtypes

    src = np.asarray(edge_index[0], dtype=np.int64)
    dst = np.asarray(edge_index[1], dtype=np.int64)
    w = np.asarray(edge_weight, dtype=np.float32)

    core = dst // NC_N
    drel = dst % NC_N

    deg_full = np.zeros(N, dtype=np.float64)
    np.add.at(deg_full, dst, w.astype(np.float64))
    dinv_full = (1.0 / np.sqrt(deg_full + 2.0)).astype(np.float32)

    wn = w * dinv_full[src] * dinv_full[dst]   # folded edge norm

    # permuted global table row for source node s
    src_core = src // NC_N
    src_m = src % NC_N
    row_global = src_core * NPAD + (src_m % BW) * NBLK + src_m // BW
    row_local = (src_m % BW) * NBLK + src_m // BW

    groups = [[[None] * 3 for _ in range(NBLK)] for _ in range(P)]
    for c in range(P):
        mask = core == c
        s_core, d_all, w_all = src_core[mask], drel[mask], wn[mask]
        rg, rl = row_global[mask], row_local[mask]
        blk = d_all // BW
        rel = (d_all % BW).astype(np.float32)
        is_local = s_core == c
        even = rg % 2 == 0
        for b in range(NBLK):
            mb = blk == b
            # remote edges split by table-row parity: parity is uniform
            # across cores, so the max-over-cores stream sizes stay balanced
            # (an absolute row split would skew per core); idx = row//2 fits
            # int16 since the table is viewed as [rows/2, 2H] row pairs.
            for s, ms in (
                (0, mb & is_local),
                (1, mb & ~is_local & even),
                (2, mb & ~is_local & ~even),
            ):
                idx = rl[ms] if s == 0 else rg[ms] // 2
                # sort by source row for DRAM locality
                o = np.argsort(idx, kind="stable")
                groups[c][b][s] = (
                    idx[o].astype(np.int16), rel[ms][o], w_all[ms][o],
                )

    nmax = np.zeros((3, NBLK), dtype=np.int64)
    for b in range(NBLK):
        for s in range(3):
            for c in range(P):
                nmax[s, b] = max(nmax[s, b], len(groups[c][b][s][0]))

    O, ST, NT, uses, meta_of, TT = _layout(nmax)

    per_core = []
    for c in range(P):
        dinv_c = np.zeros(NPAD, dtype=np.float32)
        dinv_c[:NC_N] = dinv_full[c * NC_N : (c + 1) * NC_N]
        selfw = 2.0 * dinv_c * dinv_c

        idx_bufs = [np.zeros(NT[s] * 128, dtype=np.int16) for s in range(3)]
        # meta[e] = (dst_rel, w) per consumption-order tile USE: per block,
        # the self tile first, then each stream tile overlapping the block's
        # dense slot range; out-of-range rows keep w=0.
        meta_rel = np.zeros(TT * 128, dtype=np.float32)
        meta_w = np.zeros(TT * 128, dtype=np.float32)
        om = 0
        iota128 = np.arange(128, dtype=np.float32)
        for b in range(NBLK):
            meta_rel[om * 128 : om * 128 + 128] = iota128
            meta_w[om * 128 : om * 128 + 128] = selfw[b * 128 : (b + 1) * 128]
            om += 1
            for s, T in uses[b]:
                idx, rel, ww = groups[c][b][s]
                n = len(idx)
                o_sb = int(O[s][b])
                if T == o_sb // 128:
                    idx_bufs[s][o_sb : o_sb + n] = idx
                lo_s = max(o_sb, 128 * T)
                hi_s = min(o_sb + n, 128 * (T + 1))
                if hi_s > lo_s:
                    mo = om * 128 + (lo_s - 128 * T)
                    meta_rel[mo : mo + hi_s - lo_s] = rel[lo_s - o_sb : hi_s - o_sb]
                    meta_w[mo : mo + hi_s - lo_s] = ww[lo_s - o_sb : hi_s - o_sb]
                om += 1

        # wrapped int16 index layout: idx i lives at [i % 16, i // 16],
        # replicated 8x along partitions (one stripe per Q7 core)
        wraps = [
            np.ascontiguousarray(np.tile(ib.reshape(-1, 16).T, (8, 1)))
            if len(ib)
            else np.zeros((128, 0), dtype=np.int16)
            for ib in idx_bufs
        ]
        # meta in partition-major tile layout: edge t*128+p -> [p, 2t + {0,1}]
        meta = np.empty((128, 2 * TT), dtype=np.float32)
        meta[:, 0::2] = meta_rel.reshape(TT, 128).T
        meta[:, 1::2] = meta_w.reshape(TT, 128).T

        per_core.append(
            {
                "lcidx": wraps[0],
                "loidx": wraps[1],
                "hiidx": wraps[2],
                "meta": meta,
            }
        )

    return nmax, per_core


def _build_program(tcnt, single_core=False):
    # single_core=True swaps the AllGather for a local DMA copy and builds a
    # 1-device module, so the cost-model TimelineSim (single-core only) can
    # profile the kernel; numerics of remote nodes are wrong in that mode.
    nmax = tcnt
    O, ST, NT, uses, meta_of, TT = _layout(nmax)
    TS = NT
    nc = bacc.Bacc(
        "TRN2",
        target_bir_lowering=False,
        debug=False,
        enable_asserts=False,
        num_devices=1 if single_core else P,
    )

    # ---- I/O -------------------------------------------------------------
    xT_d = nc.dram_tensor("xT", [IN_D, NC_N], BF, kind="ExternalInput")
    lcidx_d = nc.dram_tensor("lcidx", [128, max(TS[0], 1) * 8], I16, kind="ExternalInput")
    loidx_d = nc.dram_tensor("loidx", [128, max(TS[1], 1) * 8], I16, kind="ExternalInput")
    hiidx_d = nc.dram_tensor("hiidx", [128, max(TS[2], 1) * 8], I16, kind="ExternalInput")
    meta_d = nc.dram_tensor("meta", [128, 2 * TT], FP, kind="ExternalInput")
    w_d = {
        name: nc.dram_tensor(name, shape, BF, kind="ExternalInput")
        for name, shape in [
            ("W1", [IN_D, H]),
            ("W2", [H, H]),
            ("W3", [H, H]),
            ("Wg1", [H, H]),
            ("Wg2", [H, H]),
            ("Wg3", [H, H]),
            ("Wg4", [H, H]),
            ("Wh", [H, OUT_D]),
        ]
    }
    # bias columns: 0..2 = b1..b3, 3..6 = bg1..bg4, 7..13 = negated, 14 = bh
    bias_d = nc.dram_tensor("bias", [128, 24], FP, kind="ExternalInput")
    iota_d = nc.dram_tensor("iota128", [128, BW], BF, kind="ExternalInput")
    out_d = nc.dram_tensor("out", [OUT_D, NC_N], FP, kind="ExternalOutput")

    rg = [list(range(P))]

    with tile.TileContext(nc) as tc, ExitStack() as ctx:
        const = ctx.enter_context(tc.tile_pool(name="const", bufs=1))
        big = ctx.enter_context(tc.tile_pool(name="big", bufs=1))
        swp = ctx.enter_context(tc.tile_pool(name="swp", bufs=80))
        epp = ctx.enter_context(tc.tile_pool(name="epp", bufs=3))
        rp_p = ctx.enter_context(tc.tile_pool(name="rp", bufs=13))
        e2_p = ctx.enter_context(tc.tile_pool(name="e2", bufs=13))
        vlc_p = ctx.enter_context(tc.tile_pool(name="vlc", bufs=3))
        vlo_p = ctx.enter_context(tc.tile_pool(name="vlo", bufs=3))
        vhi_p = ctx.enter_context(tc.tile_pool(name="vhi", bufs=3))
        ps_dense = ctx.enter_context(tc.tile_pool(name="psd", bufs=3, space="PSUM"))
        ps_blk = ctx.enter_context(tc.tile_pool(name="psb", bufs=3, space="PSUM"))
        ps_tr = ctx.enter_context(tc.tile_pool(name="pst", bufs=2, space="PSUM"))
        dram = ctx.enter_context(tc.tile_pool(name="dram", bufs=2, space="DRAM"))

        # ---- constants ----------------------------------------------------
        def load_const(shape, src_ap, name, dtype=FP):
            t = const.tile(shape, dtype, tag=name)
            nc.sync.dma_start(t[:], src_ap)
            return t

        w_sb = {k: load_const(list(v.shape), v[:], k, BF) for k, v in w_d.items()}
        bias = load_const([128, 24], bias_d[:], "bias")
        iota = load_const([128, BW], iota_d[:], "iota", BF)
        meta_sb = load_const([128, 2 * TT], meta_d[:], "meta")
        idx_sb = [
            load_const([128, max(TS[s], 1) * 8], d[:], f"idx{s}", I16)
            for s, d in enumerate((lcidx_d, loidx_d, hiidx_d))
        ]

        h_sb = big.tile([128, NPAD], BF, tag="h")
        tt_sb = big.tile([128, NPAD], BF, tag="tt")
        xc = big.tile([IN_D, NPAD], BF, tag="xc")
        oc = big.tile([OUT_D, NPAD], FP, tag="oc")

        nc.vector.memset(xc[:, NC_N:], 0.0)
        nc.sync.dma_start(xc[:, :NC_N], xT_d[:])

        def sw_tile(g):
            """[128 edge, BW dst] one-hot(dst_rel)*w scatter tile for
            consumption-order tile g, built on the vector engine."""
            sw = swp.tile([128, BW], BF, tag="sw")
            nc.vector.tensor_scalar(
                sw[:],
                iota[:],
                meta_sb[:, 2 * g : 2 * g + 1],
                meta_sb[:, 2 * g + 1 : 2 * g + 2],
                ALU.is_equal,
                ALU.mult,
            )
            return sw

        def emit_t(b0, b1, wg):
            # t (node-major) = h_blk^T @ Wg per block; lhsT = h slice puts
            # nodes on the output partition axis, so no transposes needed.
            for b in range(b0, b1):
                trp = ps_tr.tile([128, 128], FP, tag="tr")
                nc.tensor.matmul(trp[:], h_sb[:, b * BW : (b + 1) * BW], wg[:])
                nc.scalar.activation(
                    tt_sb[:, b * BW : (b + 1) * BW], trp[:], ACT_F.Copy
                )

        # ---- embedding MLP -------------------------------------------------

        # ELU via  elu(z) = min(exp(z), 1) + max(z-1, -1)  (exact for all z):
        # exp on ACT, the shifted relu and combine on DVE.  Stage-split loops
        # keep each in-order engine queue free of cross-chunk dependency
        # chains.  The W3 combine loop interleaves layer 1's t-matmuls so the
        # first GCN table emit isn't serialized behind the whole MLP.
        for wname, bcol in [("W1", 0), ("W2", 1), ("W3", 2)]:
            # W1/W2 store h+1 = min(exp(z),1) + relu(z): both pieces come off
            # ACT and the offset is pre-folded into the next bias.  W3 output
            # feeds the GCN, so it uses the exact form with the shifted relu
            # on DVE.
            offset_form = wname != "W3"
            rps, e2s = [], []
            for off, cw in CHUNKS:
                ps = ps_dense.tile([128, 512], FP, tag="dense")
                if wname == "W1":
                    nc.tensor.matmul(
                        ps[:, :cw], w_sb["W1"][:IN_D, :], xc[:IN_D, off : off + cw]
                    )
                else:
                    nc.tensor.matmul(
                        ps[:, :cw], w_sb[wname][:], h_sb[:, off : off + cw]
                    )
                rp = rp_p.tile([128, 512], BF, tag="rp")
                if offset_form:
                    nc.scalar.activation(
                        rp[:, :cw], ps[:, :cw], ACT_F.Relu,
                        bias=bias[:, bcol : bcol + 1],
                    )
                else:
                    nc.vector.tensor_scalar(
                        rp[:, :cw], ps[:, :cw], bias[:, bcol + 16 : bcol + 17],
                        -1.0, ALU.add, ALU.max,
                    )
                e2 = e2_p.tile([128, 512], BF, tag="e2")
                nc.scalar.activation(
                    e2[:, :cw], ps[:, :cw], ACT_F.Exp,
                    bias=bias[:, bcol : bcol + 1],
                )
                rps.append(rp)
                e2s.append(e2)
            for ci, (off, cw) in enumerate(CHUNKS):
                nc.vector.scalar_tensor_tensor(
                    h_sb[:, off : off + cw], e2s[ci][:, :cw], 1.0,
                    rps[ci][:, :cw], ALU.min, ALU.add,
                )
                if wname == "W3":
                    emit_t(off // BW, min(NBLK, (off + cw) // BW), w_sb["Wg1"])

        # ---- GCN layers ---------------------------------------------------
        cstarts = []
        for s in range(3):
            sizes, rem = [], NT[s]
            while rem > C_TILES + C_TILES // 2:
                sizes.append(C_TILES)
                rem -= C_TILES
            if rem > C_TILES // 2:
                sizes.extend([(rem + 1) // 2, rem // 2])
            elif rem:
                sizes.append(rem)
            cstarts.append(np.concatenate([[0], np.cumsum(sizes)]).astype(int))
        n_chunk = [len(cstarts[s]) - 1 for s in range(3)]
        # emit gather chunks interleaved by first consuming block
        chunk_order = sorted(
            (max(0, int(np.searchsorted(O[s], int(cstarts[s][ci]) * 128,
                                        "right")) - 1), s, ci)
            for s in range(3) for ci in range(n_chunk[s])
        )

        for layer in range(4):
            wg = w_sb[f"Wg{layer + 1}"]
            bcol = 3 + layer

            if layer > 0:
                emit_t(0, NBLK, wg)

            # one flat table emit; rows are host-side permuted.  tfull is
            # viewed as [row-pairs, 2H] so the even/odd gather streams can
            # address all P*NPAD rows with int16 pair indices + elem_step.
            agin = dram.tile([NPAD, H], BF, tag="agin")
            tfull = dram.tile(
                [P * NPAD // 2, 2 * H], BF, tag="tfull", addr_space="Shared"
            )
            nc.sync.dma_start(agin[:, :], tt_sb[:])

            if single_core:
                nc.sync.dma_start(tfull[: NPAD // 2, :], agin[:, :])
            else:
                nc.gpsimd.collective_compute(
                    "AllGather",
                    ALU.bypass,
                    replica_groups=rg,
                    ins=[agin[:]],
                    outs=[tfull[:]],
                )

            tables = (agin[:, :], tfull[:, :H], tfull[:, H:])
            steps = (None, 2 * H, 2 * H)
            vpools = (vlc_p, vlo_p, vhi_p)
            # chunked gathers over the dense slot streams; the stream-final
            # call skips trailing slots via num_idxs, so its last tile is
            # memset first (w=0 matmul columns must not hit NaN garbage)
            vchunks = [[None] * n_chunk[s] for s in range(3)]
            for _, s, ci in chunk_order:
                t0 = int(cstarts[s][ci])
                nt = int(cstarts[s][ci + 1]) - t0
                nidx = min(nt * 128, ST[s] - t0 * 128)
                v = vpools[s].tile([128, C_TILES, 128], BF, tag=f"v{s}")
                if nidx < nt * 128:
                    nc.vector.memset(v[:, nt - 1, :], 0.0)
                nc.gpsimd.dma_gather(
                    v[:, :nt, :], tables[s],
                    idx_sb[s][:, t0 * 8 : (t0 + nt) * 8],
                    nidx, nidx, H, elem_step=steps[s], single_packet=False,
                )
                vchunks[s][ci] = v

            # per-block scatter-accumulate + epilogue; the self tile (SBUF
            # node-major t) leads each block's accumulation group.
            for b in range(NBLK):
                ntile = 1 + len(uses[b])
                agg = ps_blk.tile([128, BW], FP, tag="agg")
                sw = sw_tile(meta_of[b])
                nc.tensor.matmul(
                    agg[:], tt_sb[:, b * BW : (b + 1) * BW], sw[:],
                    start=True, stop=(ntile == 1),
                )
                for t, (s, T) in enumerate(uses[b], start=1):
                    sw = sw_tile(meta_of[b] + t)
                    ci = int(np.searchsorted(cstarts[s], T, "right")) - 1
                    v = vchunks[s][ci][:, T - int(cstarts[s][ci]), :]
                    nc.tensor.matmul(
                        agg[:], v, sw[:],
                        start=False, stop=(t == ntile - 1),
                    )
                rp = epp.tile([128, BW], FP, tag="rpb")
                nc.vector.tensor_scalar(
                    rp[:], agg[:], bias[:, bcol + 16 : bcol + 17],
                    -1.0, ALU.add, ALU.max,
                )
                eb = epp.tile([128, BW], BF, tag="eb")
                nc.scalar.activation(
                    eb[:], agg[:], ACT_F.Exp, bias=bias[:, bcol : bcol + 1]
                )
                nc.vector.scalar_tensor_tensor(
                    h_sb[:, b * BW : (b + 1) * BW],
                    eb[:], 1.0, rp[:], ALU.min, ALU.add,
                )

        # ---- head ----------------------------------------------------------
        for off, cw in CHUNKS:
            cw = min(cw, NC_N - off)
            ps = ps_dense.tile([128, 512], FP, tag="dense")
            nc.tensor.matmul(
                ps[:OUT_D, :cw], w_sb["Wh"][:], h_sb[:, off : off + cw]
            )
            nc.scalar.activation(
                oc[:, off : off + cw], ps[:OUT_D, :cw], ACT_F.Identity,
                bias=bias[:OUT_D, 14:15],
            )
        nc.sync.dma_start(out_d[:, :], oc[:, :NC_N])

    nc.compile()
    return nc


def _make_in_maps(inputs, per_core):
    import ml_dtypes

    x = np.asarray(inputs["x"], dtype=np.float32)
    # the W1/W2 MLP layers store h+1 (ELU plus one); the constant offset is
    # folded into the consuming layer's bias via column sums of the bf16
    # weights actually used on device
    w2bf = np.asarray(inputs["W2"], np.float32).astype(ml_dtypes.bfloat16)
    w3bf = np.asarray(inputs["W3"], np.float32).astype(ml_dtypes.bfloat16)
    bias = np.zeros((128, 24), dtype=np.float32)
    for j, nm in enumerate(["b1", "b2", "b3", "bg1", "bg2", "bg3", "bg4"]):
        b = np.asarray(inputs[nm], dtype=np.float32)
        bias[:, j] = b
        bias[:, j + 16] = b - 1.0
    bias[:, 1] -= w2bf.astype(np.float32).sum(axis=0)
    bias[:, 2] -= w3bf.astype(np.float32).sum(axis=0)
    bias[:, 17] = bias[:, 1] - 1.0
    bias[:, 18] = bias[:, 2] - 1.0
    bias[:OUT_D, 14] = np.asarray(inputs["bh"], dtype=np.float32)

    shared = {
        "bias": bias,
        "iota128": np.tile(
            np.arange(BW, dtype=np.float32), (128, 1)
        ).astype(ml_dtypes.bfloat16),
    }
    for nm in ["W1", "W2", "W3", "Wg1", "Wg2", "Wg3", "Wg4", "Wh"]:
        shared[nm] = np.ascontiguousarray(
            np.asarray(inputs[nm], np.float32)
        ).astype(ml_dtypes.bfloat16)

    in_maps = []
    for c in range(P):
        m = dict(shared)
        m["xT"] = np.ascontiguousarray(
            x[c * NC_N : (c + 1) * NC_N].T
        ).astype(ml_dtypes.bfloat16)
        m.update(per_core[c])
        in_maps.append(m)
    return in_maps


def run(inputs, trace=False):
    """Run the distributed kernel; returns (out [N, OUT_D] fp32, results)."""
    tcnt, per_core = _prep_edges(inputs["edge_index"], inputs["edge_weight"])
    nc = _build_program(tcnt)
    in_maps = _make_in_maps(inputs, per_core)
    res = run_bass_kernel_spmd(nc, in_maps, list(range(P)), trace=trace)
    out = np.concatenate(
        [res.results[c]["out"].T for c in range(P)], axis=0
    ).astype(np.float32)
    return out, res


def kernel(**inputs):
    out, _ = run(inputs, trace=False)
    return out
